# revision 21
# baseline (speedup 1.0000x reference)
"""Trainium2 Bass kernel for the KNet-style recurrent chain (batch=1).

Strategy (memory-bound, ~353MB fp32 weights on host):
  - ALL weights are bf16 on device: halves HBM traffic; bf16 moving
    operand runs 1 cycle/row on the PE at any width. Host-sim rel err
    of bf16 weights+activations is 3.9e-3 (gate: 2e-2).
  - Small GRU chain + small FCs REPLICATED on all 8 cores; FC2
    (W2a [46080,1152], W2b [576,46080]) tensor-parallel: each core
    takes 5760 rows of W2a / columns of W2b; host sums the 8 partials.
  - Matvecs run weight-moving on the PE:
        psum[1, N] (+)= x_chunk[K, 1].T @ W.T_chunk[K, N]
  - Chain biases are FOLDED into the weights as one extra K-row; every
    activation vector carries a literal 1.0 at position d. Activations
    (sigmoid/tanh/relu) read PSUM directly - no bias adds.
  - Weights are HOST-PACKED into exact SBUF tile layouts so every DMA
    is contiguous rows of >=2KB (DMA queues are descriptor-rate bound
    below ~2KB/descriptor). Chain groups are 16KB/partition to cut
    per-group semaphore waits.
  - FC2a contracts over [hSig | hS] P-layout chunks (k-split, no in2
    concat). The two 64-row K-tails merge into ONE 128-row chunk whose
    activation column is a free-layout concat + one transpose -> 9
    full chunks per stripe. b2a is added on the vector engine.
  - The FC2 stripe loop is software-pipelined on the PE: stripe s's
    matvec runs while stripe s-1's transposes + FC2b y512 group
    execute, so the add+relu engine hop is off the PE critical path.
    The y[512:576] slice of FC2b accumulates on the vector engine
    (scalar_tensor_tensor) with a final ones-vector matmul reduce.
  - W2a stripe DMAs are issued at chain milestones (6 during the
    chain, 6 during the FC2 phase) so the big weights stream in the
    chain's idle DMA bandwidth without delaying chain weights much.
  - PSUM: every accumulation group gets its own [1,576] slot (tag
    rotation bufs=3, 2 banks each) - no group ever shares a bank, so
    start=True can never clear a co-tenant's has_written state.
    y512/y64 keep dedicated banks (8 banks total).
"""

import sys

sys.path.insert(0, "/opt/trn_rl_repo")

import numpy as np
import ml_dtypes

NCORES = 8
H = 576                       # hidden size of all three GRUs
D2_HID, D2_IN, D2_OUT = 46080, 1152, 576
MSH = D2_HID // NCORES        # 5760 rows of W2a per core
NM2 = MSH // 128              # 45 h_fc columns per core
NSTR = 12                     # FC2a stripes: 11x512 + 1x128
W2B_GRP = 4                   # FC2b k-blocks per group: 45 = 11*4 + 1
CAP = 8192                    # bytes/partition per chain weight group

F32 = np.float32
BF = ml_dtypes.bfloat16

# chain weights: name -> (seg K sizes, m_out). Bias rides on the last
# segment's tail chunk (one extra row).
WSPECS = {
    "w5": ([24], 480), "w6": ([24], 480), "w7": ([48], 960),
    "w1": ([H], H),
    "wrz_q": ([480, H], 1152), "win_q": ([480], H), "whn_q": ([H], H),
    "wrz_sig": ([H, 480, H], 1152), "win_sig": ([H, 480], H),
    "whn_sig": ([H], H),
    "wrz_s": ([H, 960, H], 1152), "win_s": ([H, 960], H),
    "whn_s": ([H], H),
}


def _stripes():
    return [(s * 512, min(512, MSH - s * 512)) for s in range(NSTR)]


def _w2b_groups():
    return [(g * W2B_GRP, min(W2B_GRP, NM2 - g * W2B_GRP))
            for g in range((NM2 + W2B_GRP - 1) // W2B_GRP)]


def _grp(m_out):
    return max(1, CAP // (m_out * 2))


def _wplan(segs, m_out):
    """Deterministic chunk plan shared by builder and host packer."""
    g = _grp(m_out)
    full, tails = [], []
    off = 0
    last = len(segs) - 1
    for si, d in enumerate(segs):
        nb, tail = d // 128, d % 128
        for g0 in range(0, nb, g):
            gn = min(g, nb - g0)
            full.append((si, g0, gn, off))
            off += gn * m_out
        trows = tail + (1 if si == last else 0)
        if trows:
            tails.append((si, trows))
    return full, tails, off


def _ncols(d):
    return (d + 127) // 128


_CACHE = {}


class _Vec:
    """Activation vector in SBUF P-layout [128, ncols], with a literal
    1.0 stored at flat position d (row d%128, col d//128)."""

    def __init__(self, tile, d):
        self.tile = tile
        self.d = d

    def full_chunks(self):
        for c in range(self.d // 128):
            yield self.tile[0:128, c : c + 1]

    def tail_ap(self, trows):
        c = self.d // 128
        return self.tile[0:trows, c : c + 1]


def _build_program():
    import concourse.bass as bass  # noqa: F401
    from concourse import bacc, mybir
    import concourse.tile as tile

    f32 = mybir.dt.float32
    bf16 = mybir.dt.bfloat16
    AF = mybir.ActivationFunctionType
    ALU = mybir.AluOpType

    nc = bacc.Bacc(
        "TRN2", target_bir_lowering=False, debug=False, num_devices=NCORES
    )

    def din(name, shape, dt=bf16):
        return nc.dram_tensor(name, list(shape), dt, kind="ExternalInput")

    # --- activation inputs (extended with the 1.0 bias marker) ---
    d_x5 = din("x5", (25, 1))
    d_x6 = din("x6", (25, 1))
    d_obs = din("obs", (49, 1))
    d_hq = din("h_q", (128, 5))       # P-layout, 1.0 at (64, 4)
    d_hsig = din("h_sig", (128, 5))
    d_hs = din("h_s", (128, 5))
    d_hq_f = din("h_q_f", (1, H), f32)    # free-layout (elementwise)
    d_hsig_f = din("h_sig_f", (1, H), f32)
    d_hs_f = din("h_s_f", (1, H), f32)

    # --- chain weights: host-packed bf16, biases folded ---
    dwf, dwt = {}, {}
    for wname, (segs, m_out) in WSPECS.items():
        full, tails, tot = _wplan(segs, m_out)
        if tot:
            dwf[wname] = din(f"{wname}_f", (128, tot))
        for si, trows in tails:
            dwt[(wname, si)] = din(f"{wname}_t{si}", (trows, m_out))

    # --- FC2 weights: host-packed bf16, stripe/group major ---
    # per stripe: full [128, 8, nsz] (hSig c0-3 + hS c0-3) and merged
    # tail [128, nsz] (hSig rows 512:576 ; hS rows 512:576).
    d_w2af = din("w2af", (NSTR - 1, 128, 8 * 512))
    d_w2af_t = din("w2af_t", (128, 8 * 128))
    d_w2am = din("w2am", (NSTR - 1, 128, 512))
    d_w2am_t = din("w2am_t", (128, 128))
    d_b2a = din("b2a", (1, MSH), f32)
    d_w2b = din("w2b", (len(_w2b_groups()) - 1, 128, W2B_GRP * D2_OUT))
    d_w2b_t = din("w2b_t", (128, 1 * D2_OUT))

    d_y = nc.dram_tensor("y", [1, D2_OUT], f32, kind="ExternalOutput")

    with tile.TileContext(nc) as tc:
        with (
            tc.tile_pool(name="const", bufs=1) as constp,
            tc.tile_pool(name="vecs", bufs=1) as vecp,
            tc.tile_pool(name="smallw", bufs=1) as swp,
            tc.tile_pool(name="w2ap", bufs=1) as w2ap,
            tc.tile_pool(name="w2bp", bufs=1) as w2bp,
            tc.tile_pool(name="ps", bufs=1, space="PSUM") as psp,
        ):
            def load_const(dram, shape, name, dt=bf16):
                t = constp.tile(list(shape), dt, name=name, tag=name)
                nc.sync.dma_start(out=t, in_=dram[:])
                return t

            x5 = _Vec(load_const(d_x5, (25, 1), "t_x5"), 24)
            x6 = _Vec(load_const(d_x6, (25, 1), "t_x6"), 24)
            obs = _Vec(load_const(d_obs, (49, 1), "t_obs"), 48)
            h_q = _Vec(load_const(d_hq, (128, 5), "t_hq"), H)
            h_sig = _Vec(load_const(d_hsig, (128, 5), "t_hsig"), H)
            h_s = _Vec(load_const(d_hs, (128, 5), "t_hs"), H)
            hf = {
                "q": load_const(d_hq_f, (1, H), "t_hq_f", f32),
                "sig": load_const(d_hsig_f, (1, H), "t_hsig_f", f32),
                "s": load_const(d_hs_f, (1, H), "t_hs_f", f32),
            }
            ident = constp.tile([1, 1], f32, name="ident", tag="ident")
            nc.vector.memset(ident, 1.0)
            ones = constp.tile([128, 1], f32, name="ones", tag="ones")
            nc.vector.memset(ones, 1.0)
            h_fc = constp.tile([128, NM2], bf16, name="h_fc", tag="h_fc")
            acc64 = constp.tile([128, 64], f32, name="acc64", tag="acc64")
            nc.vector.memset(acc64, 0.0)

            def ps_alloc(name):
                return psp.tile([1, 576], f32, name=name, tag="mvb",
                                bufs=3)

            def ps_t_alloc(name):
                return psp.tile([128, 9], f32, name=name, tag="mvb",
                                bufs=3)

            def matvec(wname, segs, m_out):
                """PE matvec over P-layout segs (bias folded into the
                weights). Returns [(psum_ap, m0, width)] slot list: each
                576-wide output slot is its own PSUM allocation, so no
                accumulation group ever shares a bank."""
                full, tails, _ = _wplan([v.d for v in segs], m_out)
                chunks = []          # (wt_ap, rhs_ap, ksz)
                for si, g0, gn, foff in full:
                    wt = swp.tile([128, gn, m_out], bf16, tag="sw",
                                  name=f"w_{wname}_{si}_{g0}", bufs=5)
                    nc.sync.dma_start(
                        out=wt, in_=dwf[wname][:, foff : foff + gn * m_out]
                    )
                    fc = list(segs[si].full_chunks())
                    for b in range(gn):
                        chunks.append((wt[:, b, :], fc[g0 + b], 128))
                for si, trows in tails:
                    wtt = swp.tile([trows, m_out], bf16, tag="sw",
                                   name=f"w_{wname}_t{si}", bufs=5)
                    nc.sync.dma_start(out=wtt, in_=dwt[(wname, si)][:])
                    chunks.append((wtt, segs[si].tail_ap(trows), trows))

                nch = len(chunks)
                slots = []
                for m0 in range(0, m_out, 576):
                    w = min(576, m_out - m0)
                    slots.append((ps_alloc(f"ps_{wname}_{m0}"), m0, w))
                for ci, (wt_ap, rhs, ksz) in enumerate(chunks):
                    for ps, m0, w in slots:
                        for n0 in range(0, w, 512):
                            nsz = min(512, w - n0)
                            nc.tensor.matmul(
                                ps[0:1, n0 : n0 + nsz],
                                rhs,
                                wt_ap[0:ksz, m0 + n0 : m0 + n0 + nsz],
                                start=(ci == 0),
                                stop=(ci == nch - 1),
                                skip_group_check=True,
                            )
                return slots

            def to_play(free_ap, d, name):
                """transpose free-layout [1, d] -> P-layout [128, ncols]
                bf16 tile with a 1.0 planted at position d."""
                n_m = _ncols(d)
                cols = d // 128
                tl = d % 128
                ps_t = ps_t_alloc(f"pst_{name}")
                for c in range(n_m):
                    csz = min(128, d - c * 128)
                    nc.tensor.matmul(
                        ps_t[0:csz, c : c + 1],
                        free_ap[0:1, c * 128 : c * 128 + csz],
                        ident,
                        is_transpose=True,
                        start=(c == 0),
                        stop=(c == n_m - 1),
                        skip_group_check=True,
                    )
                pl = vecp.tile([128, n_m], bf16, name=name, tag=name)
                nc.vector.tensor_copy(pl, ps_t[:, 0:n_m])
                nc.vector.memset(pl[tl : tl + 1, cols : cols + 1], 1.0)
                return _Vec(pl, d)

            def gru(g, x_segs, h, out_name):
                (ps_r, _, _), (ps_z, _, _) = matvec(
                    f"wrz_{g}", x_segs + [h], 2 * H)
                (ps_ghn, _, _), = matvec(f"whn_{g}", [h], H)
                (ps_gin, _, _), = matvec(f"win_{g}", x_segs, H)
                rz = vecp.tile([1, 2 * H], f32, name=f"rz_{g}", tag="rz")
                nc.scalar.activation(rz[0:1, 0:H], ps_r[0:1, 0:H],
                                     AF.Sigmoid)
                nc.scalar.activation(rz[0:1, H : 2 * H], ps_z[0:1, 0:H],
                                     AF.Sigmoid)
                # n = tanh(gin + r*ghn); h' = (1-z)*n + z*h, with z*h and
                # (1-z) computed on the vector engine while tanh runs.
                t3 = vecp.tile([1, H], f32, name=f"t3_{g}", tag="t3")
                nc.vector.tensor_mul(t3, rz[0:1, 0:H], ps_ghn[0:1, 0:H])
                nc.vector.tensor_add(t3, t3, ps_gin[0:1, 0:H])
                n_t = vecp.tile([1, H], f32, name=f"n_{g}", tag="n_t")
                nc.scalar.activation(n_t, t3, AF.Tanh)
                zh = vecp.tile([1, H], f32, name=f"zh_{g}", tag="zh")
                nc.vector.tensor_mul(zh, rz[0:1, H : 2 * H], hf[g])
                omz = vecp.tile([1, H], f32, name=f"omz_{g}", tag="omz")
                nc.vector.tensor_scalar(omz, rz[0:1, H : 2 * H], -1.0, 1.0,
                                        op0=ALU.mult, op1=ALU.add)
                hn = vecp.tile([1, H], f32, name=out_name, tag=out_name)
                nc.vector.tensor_mul(hn, n_t, omz)
                nc.vector.tensor_add(hn, hn, zh)
                return hn

            def relu_mv(wname, segs, m_out, name):
                slots = matvec(wname, segs, m_out)
                out = vecp.tile([1, m_out], f32, name=name, tag="vf",
                                bufs=2)
                for ps, m0, w in slots:
                    nc.scalar.activation(out[0:1, m0 : m0 + w],
                                         ps[0:1, 0:w], AF.Relu)
                return out

            b2a_sb = constp.tile([1, MSH], f32, name="b2a_sb", tag="b2a")
            nc.sync.dma_start(out=b2a_sb, in_=d_b2a[:])
            groups = _w2b_groups()
            w2a_tiles = []
            stripes_l = _stripes()

            def issue_w2a(n):
                for _ in range(n):
                    s = len(w2a_tiles)
                    if s >= NSTR:
                        return
                    m0, nsz = stripes_l[s]
                    last = nsz != 512
                    wtf = w2ap.tile([128, 8, nsz], bf16, tag="w2a",
                                    name=f"w2af_{s}", bufs=7)
                    nc.sync.dma_start(
                        out=wtf, in_=(d_w2af_t if last else d_w2af[s])[:])
                    wtm = w2ap.tile([128, nsz], bf16, tag="w2am",
                                    name=f"w2am_{s}", bufs=7)
                    nc.sync.dma_start(
                        out=wtm, in_=(d_w2am_t if last else d_w2am[s])[:])
                    w2a_tiles.append((wtf, wtm))

            # ---- the serial chain ----
            # out6/out7 depend only on kernel inputs: compute them first
            # to fill the startup window and empty the inter-GRU gaps.
            out5_f = relu_mv("w5", [x5], 480, "out5_f")
            out5 = to_play(out5_f, 480, "out5")
            out6_f = relu_mv("w6", [x6], 480, "out6_f")
            out6 = to_play(out6_f, 480, "out6")
            out7_f = relu_mv("w7", [obs], 960, "out7_f")
            out7 = to_play(out7_f, 960, "out7")
            hQ_f = gru("q", [out5], h_q, "hQ_f")
            hQ = to_play(hQ_f, H, "hQ")
            issue_w2a(2)
            hSig_f = gru("sig", [hQ, out6], h_sig, "hSig_f")
            hSig = to_play(hSig_f, H, "hSig")
            issue_w2a(1)
            out1_f = relu_mv("w1", [hSig], H, "out1_f")
            out1 = to_play(out1_f, H, "out1")
            issue_w2a(2)
            hS_f = gru("s", [out1, out7], h_s, "hS_f")
            hS = to_play(hS_f, H, "hS")
            issue_w2a(12)

            # merged FC2a tail column: [hSig 512:576 ; hS 512:576]
            tmf = vecp.tile([1, 128], f32, name="tails_f", tag="tails_f")
            nc.vector.tensor_copy(tmf[0:1, 0:64], hSig_f[0:1, 512:576])
            nc.vector.tensor_copy(tmf[0:1, 64:128], hS_f[0:1, 512:576])
            ps_m = ps_t_alloc("pst_merge")
            nc.tensor.matmul(ps_m[0:128, 0:1], tmf, ident,
                             is_transpose=True, start=True, stop=True,
                             skip_group_check=True)
            tails_pl = vecp.tile([128, 1], bf16, name="tails_pl",
                                 tag="tails_pl")
            nc.vector.tensor_copy(tails_pl, ps_m[:, 0:1])

            # ---- FC2a stripes + FC2b, software-pipelined on the PE:
            # stripe s's matvec runs while stripe s-1's transposes and
            # FC2b group execute (hstr hop latency is off the PE path).
            hsig_cols = list(hSig.full_chunks())
            hs_cols = list(hS.full_chunks())
            ps_y512 = psp.tile([1, 512], f32, name="ps_y512", tag="y512",
                               bufs=1)
            ps_y64 = psp.tile([1, 64], f32, name="ps_y64", tag="y64",
                              bufs=1)
            w2b_tiles = {}
            hstrs = {}

            def fc2b_group(g):
                kb0, kn = groups[g]
                wtg = w2b_tiles[g]
                for j in range(kn):
                    kb = kb0 + j
                    nc.tensor.matmul(
                        ps_y512[0:1, :], h_fc[:, kb : kb + 1],
                        wtg[:, j, 0:512],
                        start=(kb == 0), stop=(kb == NM2 - 1),
                        skip_group_check=True,
                    )
                    nc.vector.scalar_tensor_tensor(
                        acc64, wtg[:, j, 512:576],
                        h_fc[:, kb : kb + 1], acc64,
                        op0=ALU.mult, op1=ALU.add,
                    )

            def finish_stripe(s):
                m0, nsz = stripes_l[s]
                ncol = nsz // 128
                hstr = hstrs.pop(s)
                ps_t = ps_t_alloc(f"pst_s{s}")
                for c in range(ncol):
                    nc.tensor.matmul(
                        ps_t[:, c : c + 1],
                        hstr[0:1, c * 128 : (c + 1) * 128],
                        ident,
                        is_transpose=True,
                        start=(c == 0),
                        stop=(c == ncol - 1),
                        skip_group_check=True,
                    )
                col0 = m0 // 128
                nc.vector.tensor_copy(
                    h_fc[:, col0 : col0 + ncol], ps_t[:, 0:ncol]
                )
                fc2b_group(s)

            for s, (m0, nsz) in enumerate(_stripes()):
                wtf, wtm = w2a_tiles[s]
                # prefetch this stripe's w2b group
                kb0, kn = groups[s]
                wtg = w2bp.tile([128, kn, D2_OUT], bf16, tag="w2b",
                                name=f"w2b_{s}", bufs=4)
                nc.sync.dma_start(
                    out=wtg, in_=(d_w2b[s] if kn == W2B_GRP
                                  else d_w2b_t)[:])
                w2b_tiles[s] = wtg

                psf = ps_alloc(f"ps_f{s}")
                rhs_list = (
                    [(wtf[:, c, :], hsig_cols[c], 128) for c in range(4)]
                    + [(wtf[:, 4 + c, :], hs_cols[c], 128) for c in range(4)]
                    + [(wtm, tails_pl[0:128, 0:1], 128)]
                )
                for ci, (wt_ap, rhs, ksz) in enumerate(rhs_list):
                    nc.tensor.matmul(
                        psf[0:1, 0:nsz],
                        rhs,
                        wt_ap[0:ksz, 0:nsz],
                        start=(ci == 0),
                        stop=(ci == 8),
                        skip_group_check=True,
                    )
                hstr = vecp.tile([1, 512], f32, name=f"hstr_{s}",
                                 tag="hstr", bufs=2)
                nc.vector.tensor_add(hstr[0:1, 0:nsz], psf[0:1, 0:nsz],
                                     b2a_sb[0:1, m0 : m0 + nsz])
                nc.scalar.activation(hstr[0:1, 0:nsz], hstr[0:1, 0:nsz],
                                     AF.Relu)
                hstrs[s] = hstr
                if s > 0:
                    finish_stripe(s - 1)
            finish_stripe(NSTR - 1)

            nc.tensor.matmul(ps_y64[0:1, :], ones, acc64,
                             start=True, stop=True, skip_group_check=True)
            y_sb = constp.tile([1, D2_OUT], f32, name="y_sb", tag="y_sb")
            nc.vector.tensor_copy(y_sb[:, 0:512], ps_y512)
            nc.vector.tensor_copy(y_sb[:, 512:576], ps_y64)
            nc.sync.dma_start(out=d_y[:], in_=y_sb)

    nc.compile()
    return nc


def _get_program():
    if "nc" not in _CACHE:
        _CACHE["nc"] = _build_program()
    return _CACHE["nc"]


# ----------------------------------------------------------------------------
# host-side data prep
# ----------------------------------------------------------------------------


def _play_ext(v, ncols):
    """vector + trailing 1.0 -> P-layout [128, ncols] bf16."""
    v = np.concatenate([np.asarray(v, F32).ravel(), [1.0]])
    buf = np.zeros((ncols, 128), F32)
    buf.reshape(-1)[: v.size] = v
    return np.ascontiguousarray(buf.T).astype(BF)


def _pack_w(wt, segs, m_out, bias):
    """Pack W.T [K, m_out] fp32 + bias into (flat [128, tot] bf16,
    {seg_idx: tail bf16}); bias row on the last segment's tail."""
    full, tails, tot = _wplan(segs, m_out)
    wt = np.asarray(wt, F32)
    bias = np.asarray(bias, F32).reshape(1, m_out)
    flat = np.empty((128, tot), BF) if tot else None
    seg_off = np.concatenate([[0], np.cumsum(segs)]).astype(int)
    last = len(segs) - 1
    for si, g0, gn, off in full:
        ro = seg_off[si] + g0 * 128
        blk = wt[ro : ro + gn * 128].reshape(gn, 128, m_out)
        flat[:, off : off + gn * m_out] = (
            blk.transpose(1, 0, 2).reshape(128, gn * m_out).astype(BF)
        )
    tail_arrs = {}
    for si, trows in tails:
        ro = seg_off[si] + (segs[si] // 128) * 128
        if si == last:
            blk = np.concatenate([wt[ro : seg_off[si + 1]], bias], axis=0)
        else:
            blk = wt[ro : ro + trows]
        tail_arrs[si] = np.ascontiguousarray(blk).astype(BF)
    return flat, tail_arrs


def _prep_inputs(inputs):
    """Build the 8 per-core input maps from the full (unsharded) inputs."""
    g = {k: np.asarray(v, F32) for k, v in inputs.items()}

    def ext(v):
        return np.concatenate(
            [np.asarray(v, F32).ravel(), [1.0]]
        ).reshape(-1, 1).astype(BF)

    common = {
        "x5": ext(g["fw_evol_diff"]),
        "x6": ext(g["fw_update_diff"]),
        "obs": ext(np.concatenate([g["obs_diff"], g["obs_innov_diff"]])),
        "h_q": _play_ext(g["h_Q"], 5),
        "h_sig": _play_ext(g["h_Sigma"], 5),
        "h_s": _play_ext(g["h_S"], 5),
        "h_q_f": g["h_Q"].reshape(1, H).copy(),
        "h_sig_f": g["h_Sigma"].reshape(1, H).copy(),
        "h_s_f": g["h_S"].reshape(1, H).copy(),
    }

    wT = {
        "w5": (g["W5"].T, g["b5"]), "w6": (g["W6"].T, g["b6"]),
        "w7": (g["W7"].T, g["b7"]), "w1": (g["W1"].T, g["b1"]),
    }
    for tag, suf in (("q", "Q"), ("sig", "Sig"), ("s", "S")):
        Wih, Whh = g[f"Wih_{suf}"], g[f"Whh_{suf}"]
        bih, bhh = g[f"bih_{suf}"], g[f"bhh_{suf}"]
        wT[f"wrz_{tag}"] = (
            np.concatenate([Wih[0 : 2 * H], Whh[0 : 2 * H]], axis=1).T,
            bih[0 : 2 * H] + bhh[0 : 2 * H],
        )
        wT[f"win_{tag}"] = (Wih[2 * H :].T, bih[2 * H :])
        wT[f"whn_{tag}"] = (Whh[2 * H :].T, bhh[2 * H :])

    for wname, (segs, m_out) in WSPECS.items():
        w, b = wT[wname]
        flat, tails = _pack_w(w, segs, m_out, b)
        if flat is not None:
            common[f"{wname}_f"] = flat
        for si, arr in tails.items():
            common[f"{wname}_t{si}"] = arr

    stripes = _stripes()
    groups = _w2b_groups()
    in_maps = []
    for k in range(NCORES):
        m = dict(common)
        sl = slice(k * MSH, (k + 1) * MSH)
        w2aT = np.ascontiguousarray(g["W2a"][sl, :].T)       # [1152, 5760]
        fulls, merged = [], []
        for s, (m0, nsz) in enumerate(stripes):
            blk = w2aT[np.r_[0:512, 576:1088], m0 : m0 + nsz]
            fulls.append(
                blk.reshape(8, 128, nsz).transpose(1, 0, 2)
                .reshape(128, 8 * nsz).astype(BF)
            )
            merged.append(np.concatenate(
                [w2aT[512:576, m0 : m0 + nsz],
                 w2aT[1088:1152, m0 : m0 + nsz]], axis=0
            ).astype(BF))
        m["w2af"] = np.stack(fulls[:-1])
        m["w2af_t"] = fulls[-1]
        m["w2am"] = np.stack(merged[:-1])
        m["w2am_t"] = merged[-1]
        m["b2a"] = g["b2a"][sl].reshape(1, -1).copy()

        w2bT = np.ascontiguousarray(g["W2b"][:, sl].T)       # [5760, 576]
        w2bG = w2bT.reshape(NM2, 128, D2_OUT)
        w2b_full = np.empty((len(groups) - 1, 128, W2B_GRP * D2_OUT), BF)
        for gi, (kb0, kn) in enumerate(groups[:-1]):
            w2b_full[gi] = (
                w2bG[kb0 : kb0 + kn].transpose(1, 0, 2)
                .reshape(128, kn * D2_OUT).astype(BF)
            )
        kb0, kn = groups[-1]
        m["w2b"] = w2b_full
        m["w2b_t"] = np.ascontiguousarray(
            w2bG[kb0 : kb0 + kn].transpose(1, 0, 2).reshape(128, kn * D2_OUT)
        ).astype(BF)
        in_maps.append(m)
    return in_maps


def run(trace=False, **inputs):
    from concourse.bass_utils import run_bass_kernel_spmd

    nc = _get_program()
    in_maps = _prep_inputs(inputs)
    res = run_bass_kernel_spmd(nc, in_maps, list(range(NCORES)), trace=trace)
    y = np.zeros(D2_OUT, np.float64)
    for r in res.results:
        y += r["y"].reshape(-1).astype(np.float64)
    out = (y.astype(F32) + np.asarray(inputs["b2b"], F32)).reshape(24, 24)
    return out, res


def kernel(**inputs):
    out, _ = run(trace=False, **inputs)
    return out


# revision 23
# speedup vs baseline: 1.1553x; 1.1553x over previous
"""Trainium2 Bass kernel for the KNet-style recurrent chain (batch=1).

Strategy (memory-bound, ~353MB fp32 weights on host):
  - ALL weights are bf16 on device: halves HBM traffic; bf16 moving
    operand runs 1 cycle/row on the PE at any width. Host-sim rel err
    of bf16 weights+activations is 3.9e-3 (gate: 2e-2).
  - Small GRU chain + small FCs REPLICATED on all 8 cores; FC2
    (W2a [46080,1152], W2b [576,46080]) tensor-parallel: each core
    takes 5760 rows of W2a / columns of W2b; host sums the 8 partials.
  - Matvecs run weight-moving on the PE:
        psum[1, N] (+)= x_chunk[K, 1].T @ W.T_chunk[K, N]
  - Chain biases are FOLDED into the weights as one extra K-row; every
    activation vector carries a literal 1.0 at position d. Activations
    (sigmoid/tanh/relu) read PSUM directly - no bias adds.
  - Weights are HOST-PACKED into exact SBUF tile layouts so every DMA
    is contiguous rows of >=2KB (DMA queues are descriptor-rate bound
    below ~2KB/descriptor). Chain groups are 16KB/partition to cut
    per-group semaphore waits.
  - FC2a contracts over [hSig | hS] P-layout chunks (k-split, no in2
    concat). The two 64-row K-tails merge into ONE 128-row chunk whose
    activation column is a free-layout concat + one transpose -> 9
    full chunks per stripe. b2a is added on the vector engine.
  - The FC2 stripe loop is software-pipelined on the PE: stripe s's
    matvec runs while stripe s-1's transposes + FC2b y512 group
    execute, so the add+relu engine hop is off the PE critical path.
    The y[512:576] slice of FC2b accumulates on the vector engine
    (scalar_tensor_tensor) with a final ones-vector matmul reduce.
  - W2a stripe DMAs are issued at chain milestones (6 during the
    chain, 6 during the FC2 phase) so the big weights stream in the
    chain's idle DMA bandwidth without delaying chain weights much.
  - PSUM: every accumulation group gets its own [1,576] slot (tag
    rotation bufs=3, 2 banks each) - no group ever shares a bank, so
    start=True can never clear a co-tenant's has_written state.
    y512/y64 keep dedicated banks (8 banks total).
"""

import sys

sys.path.insert(0, "/opt/trn_rl_repo")

import numpy as np
import ml_dtypes

NCORES = 8
H = 576                       # hidden size of all three GRUs
D2_HID, D2_IN, D2_OUT = 46080, 1152, 576
MSH = D2_HID // NCORES        # 5760 rows of W2a per core
NM2 = MSH // 128              # 45 h_fc columns per core
NSTR = 12                     # FC2a stripes: 11x512 + 1x128
W2B_GRP = 4                   # FC2b k-blocks per group: 45 = 11*4 + 1
CAP = 8192                    # bytes/partition per chain weight group

F32 = np.float32
BF = ml_dtypes.bfloat16

# chain weights: name -> (seg K sizes, m_out). Bias rides on the last
# segment's tail chunk (one extra row).
WSPECS = {
    "w5": ([24], 480), "w6": ([24], 480), "w7": ([48], 960),
    "w1": ([H], H),
    "wrz_q": ([480, H], 1152), "win_q": ([480], H), "whn_q": ([H], H),
    "wrz_sig": ([H, 480, H], 1152), "win_sig": ([H, 480], H),
    "whn_sig": ([H], H),
    "wrz_s": ([H, 960, H], 1152), "win_s": ([H, 960], H),
    "whn_s": ([H], H),
}


def _stripes():
    return [(s * 512, min(512, MSH - s * 512)) for s in range(NSTR)]


def _w2b_groups():
    return [(g * W2B_GRP, min(W2B_GRP, NM2 - g * W2B_GRP))
            for g in range((NM2 + W2B_GRP - 1) // W2B_GRP)]


def _grp(m_out):
    return max(1, CAP // (m_out * 2))


def _wplan(segs, m_out):
    """Deterministic chunk plan shared by builder and host packer."""
    g = _grp(m_out)
    full, tails = [], []
    off = 0
    last = len(segs) - 1
    for si, d in enumerate(segs):
        nb, tail = d // 128, d % 128
        for g0 in range(0, nb, g):
            gn = min(g, nb - g0)
            full.append((si, g0, gn, off))
            off += gn * m_out
        trows = tail + (1 if si == last else 0)
        if trows:
            tails.append((si, trows))
    return full, tails, off


def _ncols(d):
    return (d + 127) // 128


_CACHE = {}


class _Vec:
    """Activation vector in SBUF P-layout [128, ncols], with a literal
    1.0 stored at flat position d (row d%128, col d//128)."""

    def __init__(self, tile, d):
        self.tile = tile
        self.d = d

    def full_chunks(self):
        for c in range(self.d // 128):
            yield self.tile[0:128, c : c + 1]

    def tail_ap(self, trows):
        c = self.d // 128
        return self.tile[0:trows, c : c + 1]


def _build_program():
    import concourse.bass as bass  # noqa: F401
    from concourse import bacc, mybir
    import concourse.tile as tile

    f32 = mybir.dt.float32
    bf16 = mybir.dt.bfloat16
    AF = mybir.ActivationFunctionType
    ALU = mybir.AluOpType

    nc = bacc.Bacc(
        "TRN2", target_bir_lowering=False, debug=False, num_devices=NCORES
    )

    def din(name, shape, dt=bf16):
        return nc.dram_tensor(name, list(shape), dt, kind="ExternalInput")

    # --- activation inputs (extended with the 1.0 bias marker) ---
    d_x5 = din("x5", (25, 1))
    d_x6 = din("x6", (25, 1))
    d_obs = din("obs", (49, 1))
    d_hq = din("h_q", (128, 5))       # P-layout, 1.0 at (64, 4)
    d_hsig = din("h_sig", (128, 5))
    d_hs = din("h_s", (128, 5))
    d_hq_f = din("h_q_f", (1, H), f32)    # free-layout (elementwise)
    d_hsig_f = din("h_sig_f", (1, H), f32)
    d_hs_f = din("h_s_f", (1, H), f32)

    # --- chain weights: host-packed bf16, biases folded ---
    dwf, dwt = {}, {}
    for wname, (segs, m_out) in WSPECS.items():
        full, tails, tot = _wplan(segs, m_out)
        if tot:
            dwf[wname] = din(f"{wname}_f", (128, tot))
        for si, trows in tails:
            dwt[(wname, si)] = din(f"{wname}_t{si}", (trows, m_out))

    # --- FC2 weights: host-packed bf16, stripe/group major ---
    # per stripe: full [128, 8, nsz] (hSig c0-3 + hS c0-3) and merged
    # tail [128, nsz] (hSig rows 512:576 ; hS rows 512:576).
    d_w2af = din("w2af", (NSTR - 1, 128, 8 * 512))
    d_w2af_t = din("w2af_t", (128, 8 * 128))
    d_w2am = din("w2am", (NSTR - 1, 128, 512))
    d_w2am_t = din("w2am_t", (128, 128))
    d_b2a = din("b2a", (1, MSH), f32)
    d_w2b = din("w2b", (len(_w2b_groups()) - 1, 128, W2B_GRP * D2_OUT))
    d_w2b_t = din("w2b_t", (128, 1 * D2_OUT))

    d_y = nc.dram_tensor("y", [1, D2_OUT], f32, kind="ExternalOutput")

    with tile.TileContext(nc) as tc:
        with (
            tc.tile_pool(name="const", bufs=1) as constp,
            tc.tile_pool(name="vecs", bufs=1) as vecp,
            tc.tile_pool(name="smallw", bufs=1) as swp,
            tc.tile_pool(name="w2ap", bufs=1) as w2ap,
            tc.tile_pool(name="w2bp", bufs=1) as w2bp,
            tc.tile_pool(name="ps", bufs=1, space="PSUM") as psp,
        ):
            def load_const(dram, shape, name, dt=bf16):
                t = constp.tile(list(shape), dt, name=name, tag=name)
                nc.sync.dma_start(out=t, in_=dram[:])
                return t

            x5 = _Vec(load_const(d_x5, (25, 1), "t_x5"), 24)
            x6 = _Vec(load_const(d_x6, (25, 1), "t_x6"), 24)
            obs = _Vec(load_const(d_obs, (49, 1), "t_obs"), 48)
            h_q = _Vec(load_const(d_hq, (128, 5), "t_hq"), H)
            h_sig = _Vec(load_const(d_hsig, (128, 5), "t_hsig"), H)
            h_s = _Vec(load_const(d_hs, (128, 5), "t_hs"), H)
            hf = {
                "q": load_const(d_hq_f, (1, H), "t_hq_f", f32),
                "sig": load_const(d_hsig_f, (1, H), "t_hsig_f", f32),
                "s": load_const(d_hs_f, (1, H), "t_hs_f", f32),
            }
            ident = constp.tile([1, 1], f32, name="ident", tag="ident")
            nc.vector.memset(ident, 1.0)
            ones = constp.tile([128, 1], f32, name="ones", tag="ones")
            nc.vector.memset(ones, 1.0)
            h_fc = constp.tile([128, NM2], bf16, name="h_fc", tag="h_fc")
            acc64 = constp.tile([128, 64], f32, name="acc64", tag="acc64")
            nc.vector.memset(acc64, 0.0)

            def ps_alloc(name, tag="mvb", bufs=2):
                return psp.tile([1, 576], f32, name=name, tag=tag,
                                bufs=bufs)

            def ps_t_alloc(name):
                return psp.tile([128, 9], f32, name=name, tag="mvb",
                                bufs=2)

            def matvec(wname, segs, m_out, ps_tag="mvb", ps_bufs=2):
                """PE matvec over P-layout segs (bias folded into the
                weights). Returns [(psum_ap, m0, width)] slot list: each
                576-wide output slot is its own PSUM allocation, so no
                accumulation group ever shares a bank."""
                full, tails, _ = _wplan([v.d for v in segs], m_out)
                chunks = []          # (wt_ap, rhs_ap, ksz)
                for si, g0, gn, foff in full:
                    wt = swp.tile([128, gn, m_out], bf16, tag="sw",
                                  name=f"w_{wname}_{si}_{g0}", bufs=5)
                    nc.sync.dma_start(
                        out=wt, in_=dwf[wname][:, foff : foff + gn * m_out]
                    )
                    fc = list(segs[si].full_chunks())
                    for b in range(gn):
                        chunks.append((wt[:, b, :], fc[g0 + b], 128))
                for si, trows in tails:
                    wtt = swp.tile([trows, m_out], bf16, tag="sw",
                                   name=f"w_{wname}_t{si}", bufs=5)
                    nc.sync.dma_start(out=wtt, in_=dwt[(wname, si)][:])
                    chunks.append((wtt, segs[si].tail_ap(trows), trows))

                nch = len(chunks)
                slots = []
                for m0 in range(0, m_out, 576):
                    w = min(576, m_out - m0)
                    slots.append((ps_alloc(f"ps_{wname}_{m0}", ps_tag,
                                           ps_bufs), m0, w))
                for ci, (wt_ap, rhs, ksz) in enumerate(chunks):
                    for ps, m0, w in slots:
                        for n0 in range(0, w, 512):
                            nsz = min(512, w - n0)
                            nc.tensor.matmul(
                                ps[0:1, n0 : n0 + nsz],
                                rhs,
                                wt_ap[0:ksz, m0 + n0 : m0 + n0 + nsz],
                                start=(ci == 0),
                                stop=(ci == nch - 1),
                                skip_group_check=True,
                            )
                return slots

            def to_play(free_ap, d, name):
                """transpose free-layout [1, d] -> P-layout [128, ncols]
                bf16 tile with a 1.0 planted at position d."""
                n_m = _ncols(d)
                cols = d // 128
                tl = d % 128
                ps_t = ps_t_alloc(f"pst_{name}")
                for c in range(n_m):
                    csz = min(128, d - c * 128)
                    nc.tensor.matmul(
                        ps_t[0:csz, c : c + 1],
                        free_ap[0:1, c * 128 : c * 128 + csz],
                        ident,
                        is_transpose=True,
                        start=(c == 0),
                        stop=(c == n_m - 1),
                        skip_group_check=True,
                    )
                pl = vecp.tile([128, n_m], bf16, name=name, tag=name)
                nc.vector.tensor_copy(pl, ps_t[:, 0:n_m])
                nc.vector.memset(pl[tl : tl + 1, cols : cols + 1], 1.0)
                return _Vec(pl, d)

            def gru(g, x_segs, h, ps_ghn, out_name):
                (ps_r, _, _), (ps_z, _, _) = matvec(
                    f"wrz_{g}", x_segs + [h], 2 * H)
                (ps_gin, _, _), = matvec(f"win_{g}", x_segs, H)
                rz = vecp.tile([1, 2 * H], f32, name=f"rz_{g}", tag="rz")
                nc.scalar.activation(rz[0:1, 0:H], ps_r[0:1, 0:H],
                                     AF.Sigmoid)
                nc.scalar.activation(rz[0:1, H : 2 * H], ps_z[0:1, 0:H],
                                     AF.Sigmoid)
                # n = tanh(gin + r*ghn); h' = (1-z)*n + z*h, with z*h and
                # (1-z) computed on the vector engine while tanh runs.
                t3 = vecp.tile([1, H], f32, name=f"t3_{g}", tag="t3")
                nc.vector.tensor_mul(t3, rz[0:1, 0:H], ps_ghn[0:1, 0:H])
                nc.vector.tensor_add(t3, t3, ps_gin[0:1, 0:H])
                n_t = vecp.tile([1, H], f32, name=f"n_{g}", tag="n_t")
                nc.scalar.activation(n_t, t3, AF.Tanh)
                zh = vecp.tile([1, H], f32, name=f"zh_{g}", tag="zh")
                nc.vector.tensor_mul(zh, rz[0:1, H : 2 * H], hf[g])
                omz = vecp.tile([1, H], f32, name=f"omz_{g}", tag="omz")
                nc.vector.tensor_scalar(omz, rz[0:1, H : 2 * H], -1.0, 1.0,
                                        op0=ALU.mult, op1=ALU.add)
                hn = vecp.tile([1, H], f32, name=out_name, tag=out_name)
                nc.vector.tensor_mul(hn, n_t, omz)
                nc.vector.tensor_add(hn, hn, zh)
                return hn

            def relu_mv(wname, segs, m_out, name):
                slots = matvec(wname, segs, m_out)
                out = vecp.tile([1, m_out], f32, name=name, tag="vf",
                                bufs=2)
                for ps, m0, w in slots:
                    nc.scalar.activation(out[0:1, m0 : m0 + w],
                                         ps[0:1, 0:w], AF.Relu)
                return out

            b2a_sb = constp.tile([1, MSH], f32, name="b2a_sb", tag="b2a")
            nc.sync.dma_start(out=b2a_sb, in_=d_b2a[:])
            groups = _w2b_groups()
            w2a_tiles = []
            stripes_l = _stripes()

            def issue_w2a(n):
                for _ in range(n):
                    s = len(w2a_tiles)
                    if s >= NSTR:
                        return
                    m0, nsz = stripes_l[s]
                    last = nsz != 512
                    wtf = w2ap.tile([128, 8, nsz], bf16, tag="w2a",
                                    name=f"w2af_{s}", bufs=7)
                    nc.sync.dma_start(
                        out=wtf, in_=(d_w2af_t if last else d_w2af[s])[:])
                    wtm = w2ap.tile([128, nsz], bf16, tag="w2am",
                                    name=f"w2am_{s}", bufs=7)
                    nc.sync.dma_start(
                        out=wtm, in_=(d_w2am_t if last else d_w2am[s])[:])
                    w2a_tiles.append((wtf, wtm))

            # ---- the serial chain ----
            # out6/out7 depend only on kernel inputs: compute them first
            # to fill the startup window and empty the inter-GRU gaps.
            out5_f = relu_mv("w5", [x5], 480, "out5_f")
            out5 = to_play(out5_f, 480, "out5")
            out6_f = relu_mv("w6", [x6], 480, "out6_f")
            out6 = to_play(out6_f, 480, "out6")
            out7_f = relu_mv("w7", [obs], 960, "out7_f")
            out7 = to_play(out7_f, 960, "out7")
            # whn matvecs depend only on the constant h states: run each
            # during the PREVIOUS GRU's elementwise window (PE is idle
            # there). Their psums live in a dedicated PSUM tag so the
            # mvb rotation never pins on them.
            (ghn_q, _, _), = matvec("whn_q", [h_q], H, "ghn", 1)
            hQ_f = gru("q", [out5], h_q, ghn_q, "hQ_f")
            (ghn_sig, _, _), = matvec("whn_sig", [h_sig], H, "ghn", 1)
            hQ = to_play(hQ_f, H, "hQ")
            issue_w2a(2)
            hSig_f = gru("sig", [hQ, out6], h_sig, ghn_sig, "hSig_f")
            (ghn_s, _, _), = matvec("whn_s", [h_s], H, "ghn", 1)
            hSig = to_play(hSig_f, H, "hSig")
            issue_w2a(1)
            out1_f = relu_mv("w1", [hSig], H, "out1_f")
            out1 = to_play(out1_f, H, "out1")
            issue_w2a(2)
            hS_f = gru("s", [out1, out7], h_s, ghn_s, "hS_f")
            hS = to_play(hS_f, H, "hS")
            issue_w2a(12)

            # merged FC2a tail column: [hSig 512:576 ; hS 512:576]
            tmf = vecp.tile([1, 128], f32, name="tails_f", tag="tails_f")
            nc.vector.tensor_copy(tmf[0:1, 0:64], hSig_f[0:1, 512:576])
            nc.vector.tensor_copy(tmf[0:1, 64:128], hS_f[0:1, 512:576])
            ps_m = ps_t_alloc("pst_merge")
            nc.tensor.matmul(ps_m[0:128, 0:1], tmf, ident,
                             is_transpose=True, start=True, stop=True,
                             skip_group_check=True)
            tails_pl = vecp.tile([128, 1], bf16, name="tails_pl",
                                 tag="tails_pl")
            nc.vector.tensor_copy(tails_pl, ps_m[:, 0:1])

            # ---- FC2a stripes + FC2b, software-pipelined on the PE:
            # stripe s's matvec runs while stripe s-1's transposes and
            # FC2b group execute (hstr hop latency is off the PE path).
            hsig_cols = list(hSig.full_chunks())
            hs_cols = list(hS.full_chunks())
            ps_y512 = psp.tile([1, 512], f32, name="ps_y512", tag="y512",
                               bufs=1)
            ps_y64 = psp.tile([1, 64], f32, name="ps_y64", tag="y64",
                              bufs=1)
            w2b_tiles = {}
            hstrs = {}

            def fc2b_group(g):
                kb0, kn = groups[g]
                wtg = w2b_tiles[g]
                for j in range(kn):
                    kb = kb0 + j
                    nc.tensor.matmul(
                        ps_y512[0:1, :], h_fc[:, kb : kb + 1],
                        wtg[:, j, 0:512],
                        start=(kb == 0), stop=(kb == NM2 - 1),
                        skip_group_check=True,
                    )
                    nc.vector.scalar_tensor_tensor(
                        acc64, wtg[:, j, 512:576],
                        h_fc[:, kb : kb + 1], acc64,
                        op0=ALU.mult, op1=ALU.add,
                    )

            def finish_stripe(s):
                m0, nsz = stripes_l[s]
                ncol = nsz // 128
                hstr = hstrs.pop(s)
                ps_t = ps_t_alloc(f"pst_s{s}")
                for c in range(ncol):
                    nc.tensor.matmul(
                        ps_t[:, c : c + 1],
                        hstr[0:1, c * 128 : (c + 1) * 128],
                        ident,
                        is_transpose=True,
                        start=(c == 0),
                        stop=(c == ncol - 1),
                        skip_group_check=True,
                    )
                col0 = m0 // 128
                nc.vector.tensor_copy(
                    h_fc[:, col0 : col0 + ncol], ps_t[:, 0:ncol]
                )
                fc2b_group(s)

            for s, (m0, nsz) in enumerate(_stripes()):
                wtf, wtm = w2a_tiles[s]
                # prefetch this stripe's w2b group
                kb0, kn = groups[s]
                wtg = w2bp.tile([128, kn, D2_OUT], bf16, tag="w2b",
                                name=f"w2b_{s}", bufs=4)
                nc.sync.dma_start(
                    out=wtg, in_=(d_w2b[s] if kn == W2B_GRP
                                  else d_w2b_t)[:])
                w2b_tiles[s] = wtg

                psf = ps_alloc(f"ps_f{s}")
                rhs_list = (
                    [(wtf[:, c, :], hsig_cols[c], 128) for c in range(4)]
                    + [(wtf[:, 4 + c, :], hs_cols[c], 128) for c in range(4)]
                    + [(wtm, tails_pl[0:128, 0:1], 128)]
                )
                for ci, (wt_ap, rhs, ksz) in enumerate(rhs_list):
                    nc.tensor.matmul(
                        psf[0:1, 0:nsz],
                        rhs,
                        wt_ap[0:ksz, 0:nsz],
                        start=(ci == 0),
                        stop=(ci == 8),
                        skip_group_check=True,
                    )
                hstr = vecp.tile([1, 512], f32, name=f"hstr_{s}",
                                 tag="hstr", bufs=2)
                nc.vector.tensor_add(hstr[0:1, 0:nsz], psf[0:1, 0:nsz],
                                     b2a_sb[0:1, m0 : m0 + nsz])
                nc.scalar.activation(hstr[0:1, 0:nsz], hstr[0:1, 0:nsz],
                                     AF.Relu)
                hstrs[s] = hstr
                if s > 0:
                    finish_stripe(s - 1)
            finish_stripe(NSTR - 1)

            nc.tensor.matmul(ps_y64[0:1, :], ones, acc64,
                             start=True, stop=True, skip_group_check=True)
            y_sb = constp.tile([1, D2_OUT], f32, name="y_sb", tag="y_sb")
            nc.vector.tensor_copy(y_sb[:, 0:512], ps_y512)
            nc.vector.tensor_copy(y_sb[:, 512:576], ps_y64)
            nc.sync.dma_start(out=d_y[:], in_=y_sb)

    nc.compile()
    return nc


def _get_program():
    if "nc" not in _CACHE:
        _CACHE["nc"] = _build_program()
    return _CACHE["nc"]


# ----------------------------------------------------------------------------
# host-side data prep
# ----------------------------------------------------------------------------


def _play_ext(v, ncols):
    """vector + trailing 1.0 -> P-layout [128, ncols] bf16."""
    v = np.concatenate([np.asarray(v, F32).ravel(), [1.0]])
    buf = np.zeros((ncols, 128), F32)
    buf.reshape(-1)[: v.size] = v
    return np.ascontiguousarray(buf.T).astype(BF)


def _pack_w(wt, segs, m_out, bias):
    """Pack W.T [K, m_out] fp32 + bias into (flat [128, tot] bf16,
    {seg_idx: tail bf16}); bias row on the last segment's tail."""
    full, tails, tot = _wplan(segs, m_out)
    wt = np.asarray(wt, F32)
    bias = np.asarray(bias, F32).reshape(1, m_out)
    flat = np.empty((128, tot), BF) if tot else None
    seg_off = np.concatenate([[0], np.cumsum(segs)]).astype(int)
    last = len(segs) - 1
    for si, g0, gn, off in full:
        ro = seg_off[si] + g0 * 128
        blk = wt[ro : ro + gn * 128].reshape(gn, 128, m_out)
        flat[:, off : off + gn * m_out] = (
            blk.transpose(1, 0, 2).reshape(128, gn * m_out).astype(BF)
        )
    tail_arrs = {}
    for si, trows in tails:
        ro = seg_off[si] + (segs[si] // 128) * 128
        if si == last:
            blk = np.concatenate([wt[ro : seg_off[si + 1]], bias], axis=0)
        else:
            blk = wt[ro : ro + trows]
        tail_arrs[si] = np.ascontiguousarray(blk).astype(BF)
    return flat, tail_arrs


def _prep_inputs(inputs):
    """Build the 8 per-core input maps from the full (unsharded) inputs."""
    g = {k: np.asarray(v, F32) for k, v in inputs.items()}

    def ext(v):
        return np.concatenate(
            [np.asarray(v, F32).ravel(), [1.0]]
        ).reshape(-1, 1).astype(BF)

    common = {
        "x5": ext(g["fw_evol_diff"]),
        "x6": ext(g["fw_update_diff"]),
        "obs": ext(np.concatenate([g["obs_diff"], g["obs_innov_diff"]])),
        "h_q": _play_ext(g["h_Q"], 5),
        "h_sig": _play_ext(g["h_Sigma"], 5),
        "h_s": _play_ext(g["h_S"], 5),
        "h_q_f": g["h_Q"].reshape(1, H).copy(),
        "h_sig_f": g["h_Sigma"].reshape(1, H).copy(),
        "h_s_f": g["h_S"].reshape(1, H).copy(),
    }

    wT = {
        "w5": (g["W5"].T, g["b5"]), "w6": (g["W6"].T, g["b6"]),
        "w7": (g["W7"].T, g["b7"]), "w1": (g["W1"].T, g["b1"]),
    }
    for tag, suf in (("q", "Q"), ("sig", "Sig"), ("s", "S")):
        Wih, Whh = g[f"Wih_{suf}"], g[f"Whh_{suf}"]
        bih, bhh = g[f"bih_{suf}"], g[f"bhh_{suf}"]
        wT[f"wrz_{tag}"] = (
            np.concatenate([Wih[0 : 2 * H], Whh[0 : 2 * H]], axis=1).T,
            bih[0 : 2 * H] + bhh[0 : 2 * H],
        )
        wT[f"win_{tag}"] = (Wih[2 * H :].T, bih[2 * H :])
        wT[f"whn_{tag}"] = (Whh[2 * H :].T, bhh[2 * H :])

    for wname, (segs, m_out) in WSPECS.items():
        w, b = wT[wname]
        flat, tails = _pack_w(w, segs, m_out, b)
        if flat is not None:
            common[f"{wname}_f"] = flat
        for si, arr in tails.items():
            common[f"{wname}_t{si}"] = arr

    stripes = _stripes()
    groups = _w2b_groups()
    in_maps = []
    for k in range(NCORES):
        m = dict(common)
        sl = slice(k * MSH, (k + 1) * MSH)
        w2aT = np.ascontiguousarray(g["W2a"][sl, :].T)       # [1152, 5760]
        fulls, merged = [], []
        for s, (m0, nsz) in enumerate(stripes):
            blk = w2aT[np.r_[0:512, 576:1088], m0 : m0 + nsz]
            fulls.append(
                blk.reshape(8, 128, nsz).transpose(1, 0, 2)
                .reshape(128, 8 * nsz).astype(BF)
            )
            merged.append(np.concatenate(
                [w2aT[512:576, m0 : m0 + nsz],
                 w2aT[1088:1152, m0 : m0 + nsz]], axis=0
            ).astype(BF))
        m["w2af"] = np.stack(fulls[:-1])
        m["w2af_t"] = fulls[-1]
        m["w2am"] = np.stack(merged[:-1])
        m["w2am_t"] = merged[-1]
        m["b2a"] = g["b2a"][sl].reshape(1, -1).copy()

        w2bT = np.ascontiguousarray(g["W2b"][:, sl].T)       # [5760, 576]
        w2bG = w2bT.reshape(NM2, 128, D2_OUT)
        w2b_full = np.empty((len(groups) - 1, 128, W2B_GRP * D2_OUT), BF)
        for gi, (kb0, kn) in enumerate(groups[:-1]):
            w2b_full[gi] = (
                w2bG[kb0 : kb0 + kn].transpose(1, 0, 2)
                .reshape(128, kn * D2_OUT).astype(BF)
            )
        kb0, kn = groups[-1]
        m["w2b"] = w2b_full
        m["w2b_t"] = np.ascontiguousarray(
            w2bG[kb0 : kb0 + kn].transpose(1, 0, 2).reshape(128, kn * D2_OUT)
        ).astype(BF)
        in_maps.append(m)
    return in_maps


def run(trace=False, **inputs):
    from concourse.bass_utils import run_bass_kernel_spmd

    nc = _get_program()
    in_maps = _prep_inputs(inputs)
    res = run_bass_kernel_spmd(nc, in_maps, list(range(NCORES)), trace=trace)
    y = np.zeros(D2_OUT, np.float64)
    for r in res.results:
        y += r["y"].reshape(-1).astype(np.float64)
    out = (y.astype(F32) + np.asarray(inputs["b2b"], F32)).reshape(24, 24)
    return out, res


def kernel(**inputs):
    out, _ = run(trace=False, **inputs)
    return out


# revision 24
# speedup vs baseline: 1.1636x; 1.0072x over previous
"""Trainium2 Bass kernel for the KNet-style recurrent chain (batch=1).

Strategy (memory-bound, ~353MB fp32 weights on host):
  - ALL weights are bf16 on device: halves HBM traffic; bf16 moving
    operand runs 1 cycle/row on the PE at any width. Host-sim rel err
    of bf16 weights+activations is 3.9e-3 (gate: 2e-2).
  - Small GRU chain + small FCs REPLICATED on all 8 cores; FC2
    (W2a [46080,1152], W2b [576,46080]) tensor-parallel: each core
    takes 5760 rows of W2a / columns of W2b; host sums the 8 partials.
  - Matvecs run weight-moving on the PE:
        psum[1, N] (+)= x_chunk[K, 1].T @ W.T_chunk[K, N]
  - Chain biases are FOLDED into the weights as one extra K-row; every
    activation vector carries a literal 1.0 at position d. Activations
    (sigmoid/tanh/relu) read PSUM directly - no bias adds.
  - Weights are HOST-PACKED into exact SBUF tile layouts so every DMA
    is contiguous rows of >=2KB (DMA queues are descriptor-rate bound
    below ~2KB/descriptor). Chain groups are 16KB/partition to cut
    per-group semaphore waits.
  - FC2a contracts over [hSig | hS] P-layout chunks (k-split, no in2
    concat). The two 64-row K-tails merge into ONE 128-row chunk whose
    activation column is a free-layout concat + one transpose -> 9
    full chunks per stripe. b2a is added on the vector engine.
  - The FC2 stripe loop is software-pipelined on the PE: stripe s's
    matvec runs while stripe s-1's transposes + FC2b y512 group
    execute, so the add+relu engine hop is off the PE critical path.
    The y[512:576] slice of FC2b accumulates on the vector engine
    (scalar_tensor_tensor) with a final ones-vector matmul reduce.
  - W2a stripe DMAs are issued at chain milestones (6 during the
    chain, 6 during the FC2 phase) so the big weights stream in the
    chain's idle DMA bandwidth without delaying chain weights much.
  - PSUM: every accumulation group gets its own [1,576] slot (tag
    rotation bufs=3, 2 banks each) - no group ever shares a bank, so
    start=True can never clear a co-tenant's has_written state.
    y512/y64 keep dedicated banks (8 banks total).
"""

import sys

sys.path.insert(0, "/opt/trn_rl_repo")

import numpy as np
import ml_dtypes

NCORES = 8
H = 576                       # hidden size of all three GRUs
D2_HID, D2_IN, D2_OUT = 46080, 1152, 576
MSH = D2_HID // NCORES        # 5760 rows of W2a per core
NM2 = MSH // 128              # 45 h_fc columns per core
NSTR = 12                     # FC2a stripes: 11x512 + 1x128
W2B_GRP = 4                   # FC2b k-blocks per group: 45 = 11*4 + 1
CAP = 8192                    # bytes/partition per chain weight group

F32 = np.float32
BF = ml_dtypes.bfloat16

# chain weights: name -> (seg K sizes, m_out). Bias rides on the last
# segment's tail chunk (one extra row).
WSPECS = {
    "w5": ([24], 480), "w6": ([24], 480), "w7": ([48], 960),
    "w1": ([H], H),
    "wrz_q": ([480, H], 1152), "win_q": ([480], H), "whn_q": ([H], H),
    "wrz_sig": ([H, 480, H], 1152), "win_sig": ([H, 480], H),
    "whn_sig": ([H], H),
    "wrz_s": ([H, 960, H], 1152), "win_s": ([H, 960], H),
    "whn_s": ([H], H),
}


def _stripes():
    return [(s * 512, min(512, MSH - s * 512)) for s in range(NSTR)]


def _w2b_groups():
    return [(g * W2B_GRP, min(W2B_GRP, NM2 - g * W2B_GRP))
            for g in range((NM2 + W2B_GRP - 1) // W2B_GRP)]


def _grp(m_out):
    return max(1, CAP // (m_out * 2))


def _wplan(segs, m_out):
    """Deterministic chunk plan shared by builder and host packer."""
    g = _grp(m_out)
    full, tails = [], []
    off = 0
    last = len(segs) - 1
    for si, d in enumerate(segs):
        nb, tail = d // 128, d % 128
        for g0 in range(0, nb, g):
            gn = min(g, nb - g0)
            full.append((si, g0, gn, off))
            off += gn * m_out
        trows = tail + (1 if si == last else 0)
        if trows:
            tails.append((si, trows))
    return full, tails, off


def _ncols(d):
    return (d + 127) // 128


_CACHE = {}


class _Vec:
    """Activation vector in SBUF P-layout [128, ncols], with a literal
    1.0 stored at flat position d (row d%128, col d//128)."""

    def __init__(self, tile, d):
        self.tile = tile
        self.d = d

    def full_chunks(self):
        for c in range(self.d // 128):
            yield self.tile[0:128, c : c + 1]

    def tail_ap(self, trows):
        c = self.d // 128
        return self.tile[0:trows, c : c + 1]


def _build_program():
    import concourse.bass as bass  # noqa: F401
    from concourse import bacc, mybir
    import concourse.tile as tile

    f32 = mybir.dt.float32
    bf16 = mybir.dt.bfloat16
    AF = mybir.ActivationFunctionType
    ALU = mybir.AluOpType

    nc = bacc.Bacc(
        "TRN2", target_bir_lowering=False, debug=False, num_devices=NCORES
    )

    def din(name, shape, dt=bf16):
        return nc.dram_tensor(name, list(shape), dt, kind="ExternalInput")

    # --- activation inputs (extended with the 1.0 bias marker) ---
    d_x5 = din("x5", (25, 1))
    d_x6 = din("x6", (25, 1))
    d_obs = din("obs", (49, 1))
    d_hq = din("h_q", (128, 5))       # P-layout, 1.0 at (64, 4)
    d_hsig = din("h_sig", (128, 5))
    d_hs = din("h_s", (128, 5))
    d_hq_f = din("h_q_f", (1, H), f32)    # free-layout (elementwise)
    d_hsig_f = din("h_sig_f", (1, H), f32)
    d_hs_f = din("h_s_f", (1, H), f32)

    # --- chain weights: host-packed bf16, biases folded ---
    dwf, dwt = {}, {}
    for wname, (segs, m_out) in WSPECS.items():
        full, tails, tot = _wplan(segs, m_out)
        if tot:
            dwf[wname] = din(f"{wname}_f", (128, tot))
        for si, trows in tails:
            dwt[(wname, si)] = din(f"{wname}_t{si}", (trows, m_out))

    # --- FC2 weights: host-packed bf16, stripe/group major ---
    # per stripe: full [128, 8, nsz] (hSig c0-3 + hS c0-3) and merged
    # tail [128, nsz] (hSig rows 512:576 ; hS rows 512:576).
    d_w2af = din("w2af", (NSTR - 1, 128, 8 * 512))
    d_w2af_t = din("w2af_t", (128, 8 * 128))
    d_w2am = din("w2am", (NSTR - 1, 128, 512))
    d_w2am_t = din("w2am_t", (128, 128))
    d_b2a = din("b2a", (1, MSH), f32)
    d_w2b = din("w2b", (len(_w2b_groups()) - 1, 128, W2B_GRP * D2_OUT))
    d_w2b_t = din("w2b_t", (128, 1 * D2_OUT))

    d_y = nc.dram_tensor("y", [1, D2_OUT], f32, kind="ExternalOutput")

    with tile.TileContext(nc) as tc:
        with (
            tc.tile_pool(name="const", bufs=1) as constp,
            tc.tile_pool(name="vecs", bufs=1) as vecp,
            tc.tile_pool(name="smallw", bufs=1) as swp,
            tc.tile_pool(name="w2ap", bufs=1) as w2ap,
            tc.tile_pool(name="w2bp", bufs=1) as w2bp,
            tc.tile_pool(name="ps", bufs=1, space="PSUM") as psp,
        ):
            def load_const(dram, shape, name, dt=bf16):
                t = constp.tile(list(shape), dt, name=name, tag=name)
                nc.sync.dma_start(out=t, in_=dram[:])
                return t

            x5 = _Vec(load_const(d_x5, (25, 1), "t_x5"), 24)
            x6 = _Vec(load_const(d_x6, (25, 1), "t_x6"), 24)
            obs = _Vec(load_const(d_obs, (49, 1), "t_obs"), 48)
            h_q = _Vec(load_const(d_hq, (128, 5), "t_hq"), H)
            h_sig = _Vec(load_const(d_hsig, (128, 5), "t_hsig"), H)
            h_s = _Vec(load_const(d_hs, (128, 5), "t_hs"), H)
            hf = {
                "q": load_const(d_hq_f, (1, H), "t_hq_f", f32),
                "sig": load_const(d_hsig_f, (1, H), "t_hsig_f", f32),
                "s": load_const(d_hs_f, (1, H), "t_hs_f", f32),
            }
            ident = constp.tile([1, 1], f32, name="ident", tag="ident")
            nc.vector.memset(ident, 1.0)
            ones = constp.tile([128, 1], f32, name="ones", tag="ones")
            nc.vector.memset(ones, 1.0)
            h_fc = constp.tile([128, NM2], bf16, name="h_fc", tag="h_fc")
            acc64 = constp.tile([128, 64], f32, name="acc64", tag="acc64")
            nc.vector.memset(acc64, 0.0)

            def ps_alloc(name, tag="mvb", bufs=2):
                return psp.tile([1, 576], f32, name=name, tag=tag,
                                bufs=bufs)

            def ps_t_alloc(name):
                return psp.tile([128, 9], f32, name=name, tag="mvb",
                                bufs=2)

            def matvec(wname, segs, m_out, ps_tag="mvb", ps_bufs=2):
                """PE matvec over P-layout segs (bias folded into the
                weights). Returns [(psum_ap, m0, width)] slot list: each
                576-wide output slot is its own PSUM allocation, so no
                accumulation group ever shares a bank."""
                full, tails, _ = _wplan([v.d for v in segs], m_out)
                chunks = []          # (wt_ap, rhs_ap, ksz)
                for si, g0, gn, foff in full:
                    wt = swp.tile([128, gn, m_out], bf16, tag="sw",
                                  name=f"w_{wname}_{si}_{g0}", bufs=6)
                    nc.sync.dma_start(
                        out=wt, in_=dwf[wname][:, foff : foff + gn * m_out]
                    )
                    fc = list(segs[si].full_chunks())
                    for b in range(gn):
                        chunks.append((wt[:, b, :], fc[g0 + b], 128))
                for si, trows in tails:
                    wtt = swp.tile([trows, m_out], bf16, tag="sw",
                                   name=f"w_{wname}_t{si}", bufs=6)
                    nc.sync.dma_start(out=wtt, in_=dwt[(wname, si)][:])
                    chunks.append((wtt, segs[si].tail_ap(trows), trows))

                nch = len(chunks)
                slots = []
                for m0 in range(0, m_out, 576):
                    w = min(576, m_out - m0)
                    slots.append((ps_alloc(f"ps_{wname}_{m0}", ps_tag,
                                           ps_bufs), m0, w))
                for ci, (wt_ap, rhs, ksz) in enumerate(chunks):
                    for ps, m0, w in slots:
                        for n0 in range(0, w, 512):
                            nsz = min(512, w - n0)
                            nc.tensor.matmul(
                                ps[0:1, n0 : n0 + nsz],
                                rhs,
                                wt_ap[0:ksz, m0 + n0 : m0 + n0 + nsz],
                                start=(ci == 0),
                                stop=(ci == nch - 1),
                                skip_group_check=True,
                            )
                return slots

            def to_play(free_ap, d, name):
                """transpose free-layout [1, d] -> P-layout [128, ncols]
                bf16 tile with a 1.0 planted at position d."""
                n_m = _ncols(d)
                cols = d // 128
                tl = d % 128
                ps_t = ps_t_alloc(f"pst_{name}")
                for c in range(n_m):
                    csz = min(128, d - c * 128)
                    nc.tensor.matmul(
                        ps_t[0:csz, c : c + 1],
                        free_ap[0:1, c * 128 : c * 128 + csz],
                        ident,
                        is_transpose=True,
                        start=(c == 0),
                        stop=(c == n_m - 1),
                        skip_group_check=True,
                    )
                pl = vecp.tile([128, n_m], bf16, name=name, tag=name)
                nc.vector.tensor_copy(pl, ps_t[:, 0:n_m])
                nc.vector.memset(pl[tl : tl + 1, cols : cols + 1], 1.0)
                return _Vec(pl, d)

            def gru(g, x_segs, h, ps_ghn, out_name):
                (ps_r, _, _), (ps_z, _, _) = matvec(
                    f"wrz_{g}", x_segs + [h], 2 * H)
                (ps_gin, _, _), = matvec(f"win_{g}", x_segs, H)
                rz = vecp.tile([1, 2 * H], f32, name=f"rz_{g}", tag="rz")
                nc.scalar.activation(rz[0:1, 0:H], ps_r[0:1, 0:H],
                                     AF.Sigmoid)
                nc.scalar.activation(rz[0:1, H : 2 * H], ps_z[0:1, 0:H],
                                     AF.Sigmoid)
                # n = tanh(gin + r*ghn); h' = (1-z)*n + z*h, with z*h and
                # (1-z) computed on the vector engine while tanh runs.
                t3 = vecp.tile([1, H], f32, name=f"t3_{g}", tag="t3")
                nc.vector.tensor_mul(t3, rz[0:1, 0:H], ps_ghn[0:1, 0:H])
                nc.vector.tensor_add(t3, t3, ps_gin[0:1, 0:H])
                n_t = vecp.tile([1, H], f32, name=f"n_{g}", tag="n_t")
                nc.scalar.activation(n_t, t3, AF.Tanh)
                zh = vecp.tile([1, H], f32, name=f"zh_{g}", tag="zh")
                nc.vector.tensor_mul(zh, rz[0:1, H : 2 * H], hf[g])
                omz = vecp.tile([1, H], f32, name=f"omz_{g}", tag="omz")
                nc.vector.tensor_scalar(omz, rz[0:1, H : 2 * H], -1.0, 1.0,
                                        op0=ALU.mult, op1=ALU.add)
                hn = vecp.tile([1, H], f32, name=out_name, tag=out_name)
                nc.vector.tensor_mul(hn, n_t, omz)
                nc.vector.tensor_add(hn, hn, zh)
                return hn

            def relu_mv(wname, segs, m_out, name):
                slots = matvec(wname, segs, m_out)
                out = vecp.tile([1, m_out], f32, name=name, tag="vf",
                                bufs=2)
                for ps, m0, w in slots:
                    nc.scalar.activation(out[0:1, m0 : m0 + w],
                                         ps[0:1, 0:w], AF.Relu)
                return out

            b2a_sb = constp.tile([1, MSH], f32, name="b2a_sb", tag="b2a")
            nc.sync.dma_start(out=b2a_sb, in_=d_b2a[:])
            groups = _w2b_groups()
            w2a_tiles = []
            stripes_l = _stripes()

            def issue_w2a(n):
                for _ in range(n):
                    s = len(w2a_tiles)
                    if s >= NSTR:
                        return
                    m0, nsz = stripes_l[s]
                    last = nsz != 512
                    wtf = w2ap.tile([128, 8, nsz], bf16, tag="w2a",
                                    name=f"w2af_{s}", bufs=7)
                    nc.sync.dma_start(
                        out=wtf, in_=(d_w2af_t if last else d_w2af[s])[:])
                    wtm = w2ap.tile([128, nsz], bf16, tag="w2am",
                                    name=f"w2am_{s}", bufs=7)
                    nc.sync.dma_start(
                        out=wtm, in_=(d_w2am_t if last else d_w2am[s])[:])
                    w2a_tiles.append((wtf, wtm))

            # ---- the serial chain ----
            # out6/out7 depend only on kernel inputs: compute them first
            # to fill the startup window and empty the inter-GRU gaps.
            out5_f = relu_mv("w5", [x5], 480, "out5_f")
            out5 = to_play(out5_f, 480, "out5")
            out6_f = relu_mv("w6", [x6], 480, "out6_f")
            out6 = to_play(out6_f, 480, "out6")
            out7_f = relu_mv("w7", [obs], 960, "out7_f")
            out7 = to_play(out7_f, 960, "out7")
            # whn matvecs depend only on the constant h states: run each
            # during the PREVIOUS GRU's elementwise window (PE is idle
            # there). Their psums live in a dedicated PSUM tag so the
            # mvb rotation never pins on them.
            (ghn_q, _, _), = matvec("whn_q", [h_q], H, "ghn", 1)
            hQ_f = gru("q", [out5], h_q, ghn_q, "hQ_f")
            (ghn_sig, _, _), = matvec("whn_sig", [h_sig], H, "ghn", 1)
            hQ = to_play(hQ_f, H, "hQ")
            issue_w2a(2)
            hSig_f = gru("sig", [hQ, out6], h_sig, ghn_sig, "hSig_f")
            (ghn_s, _, _), = matvec("whn_s", [h_s], H, "ghn", 1)
            hSig = to_play(hSig_f, H, "hSig")
            issue_w2a(1)
            out1_f = relu_mv("w1", [hSig], H, "out1_f")
            out1 = to_play(out1_f, H, "out1")
            issue_w2a(2)
            hS_f = gru("s", [out1, out7], h_s, ghn_s, "hS_f")
            hS = to_play(hS_f, H, "hS")
            issue_w2a(12)

            # merged FC2a tail column: [hSig 512:576 ; hS 512:576]
            tmf = vecp.tile([1, 128], f32, name="tails_f", tag="tails_f")
            nc.vector.tensor_copy(tmf[0:1, 0:64], hSig_f[0:1, 512:576])
            nc.vector.tensor_copy(tmf[0:1, 64:128], hS_f[0:1, 512:576])
            ps_m = ps_t_alloc("pst_merge")
            nc.tensor.matmul(ps_m[0:128, 0:1], tmf, ident,
                             is_transpose=True, start=True, stop=True,
                             skip_group_check=True)
            tails_pl = vecp.tile([128, 1], bf16, name="tails_pl",
                                 tag="tails_pl")
            nc.vector.tensor_copy(tails_pl, ps_m[:, 0:1])

            # ---- FC2a stripes + FC2b, software-pipelined on the PE:
            # stripe s's matvec runs while stripe s-1's transposes and
            # FC2b group execute (hstr hop latency is off the PE path).
            hsig_cols = list(hSig.full_chunks())
            hs_cols = list(hS.full_chunks())
            ps_y512 = psp.tile([1, 512], f32, name="ps_y512", tag="y512",
                               bufs=1)
            ps_y64 = psp.tile([1, 64], f32, name="ps_y64", tag="y64",
                              bufs=1)
            w2b_tiles = {}
            hstrs = {}

            def fc2b_group(g):
                kb0, kn = groups[g]
                wtg = w2b_tiles[g]
                for j in range(kn):
                    kb = kb0 + j
                    nc.tensor.matmul(
                        ps_y512[0:1, :], h_fc[:, kb : kb + 1],
                        wtg[:, j, 0:512],
                        start=(kb == 0), stop=(kb == NM2 - 1),
                        skip_group_check=True,
                    )
                    nc.vector.scalar_tensor_tensor(
                        acc64, wtg[:, j, 512:576],
                        h_fc[:, kb : kb + 1], acc64,
                        op0=ALU.mult, op1=ALU.add,
                    )

            def finish_stripe(s):
                m0, nsz = stripes_l[s]
                ncol = nsz // 128
                hstr = hstrs.pop(s)
                ps_t = ps_t_alloc(f"pst_s{s}")
                for c in range(ncol):
                    nc.tensor.matmul(
                        ps_t[:, c : c + 1],
                        hstr[0:1, c * 128 : (c + 1) * 128],
                        ident,
                        is_transpose=True,
                        start=(c == 0),
                        stop=(c == ncol - 1),
                        skip_group_check=True,
                    )
                col0 = m0 // 128
                nc.vector.tensor_copy(
                    h_fc[:, col0 : col0 + ncol], ps_t[:, 0:ncol]
                )
                fc2b_group(s)

            for s, (m0, nsz) in enumerate(_stripes()):
                wtf, wtm = w2a_tiles[s]
                # prefetch this stripe's w2b group
                kb0, kn = groups[s]
                wtg = w2bp.tile([128, kn, D2_OUT], bf16, tag="w2b",
                                name=f"w2b_{s}", bufs=4)
                nc.sync.dma_start(
                    out=wtg, in_=(d_w2b[s] if kn == W2B_GRP
                                  else d_w2b_t)[:])
                w2b_tiles[s] = wtg

                psf = ps_alloc(f"ps_f{s}")
                rhs_list = (
                    [(wtf[:, c, :], hsig_cols[c], 128) for c in range(4)]
                    + [(wtf[:, 4 + c, :], hs_cols[c], 128) for c in range(4)]
                    + [(wtm, tails_pl[0:128, 0:1], 128)]
                )
                for ci, (wt_ap, rhs, ksz) in enumerate(rhs_list):
                    nc.tensor.matmul(
                        psf[0:1, 0:nsz],
                        rhs,
                        wt_ap[0:ksz, 0:nsz],
                        start=(ci == 0),
                        stop=(ci == 8),
                        skip_group_check=True,
                    )
                hstr = vecp.tile([1, 512], f32, name=f"hstr_{s}",
                                 tag="hstr", bufs=2)
                nc.vector.tensor_add(hstr[0:1, 0:nsz], psf[0:1, 0:nsz],
                                     b2a_sb[0:1, m0 : m0 + nsz])
                nc.scalar.activation(hstr[0:1, 0:nsz], hstr[0:1, 0:nsz],
                                     AF.Relu)
                hstrs[s] = hstr
                if s > 0:
                    finish_stripe(s - 1)
            finish_stripe(NSTR - 1)

            nc.tensor.matmul(ps_y64[0:1, :], ones, acc64,
                             start=True, stop=True, skip_group_check=True)
            y_sb = constp.tile([1, D2_OUT], f32, name="y_sb", tag="y_sb")
            nc.vector.tensor_copy(y_sb[:, 0:512], ps_y512)
            nc.vector.tensor_copy(y_sb[:, 512:576], ps_y64)
            nc.sync.dma_start(out=d_y[:], in_=y_sb)

    nc.compile()
    return nc


def _get_program():
    if "nc" not in _CACHE:
        _CACHE["nc"] = _build_program()
    return _CACHE["nc"]


# ----------------------------------------------------------------------------
# host-side data prep
# ----------------------------------------------------------------------------


def _play_ext(v, ncols):
    """vector + trailing 1.0 -> P-layout [128, ncols] bf16."""
    v = np.concatenate([np.asarray(v, F32).ravel(), [1.0]])
    buf = np.zeros((ncols, 128), F32)
    buf.reshape(-1)[: v.size] = v
    return np.ascontiguousarray(buf.T).astype(BF)


def _pack_w(wt, segs, m_out, bias):
    """Pack W.T [K, m_out] fp32 + bias into (flat [128, tot] bf16,
    {seg_idx: tail bf16}); bias row on the last segment's tail."""
    full, tails, tot = _wplan(segs, m_out)
    wt = np.asarray(wt, F32)
    bias = np.asarray(bias, F32).reshape(1, m_out)
    flat = np.empty((128, tot), BF) if tot else None
    seg_off = np.concatenate([[0], np.cumsum(segs)]).astype(int)
    last = len(segs) - 1
    for si, g0, gn, off in full:
        ro = seg_off[si] + g0 * 128
        blk = wt[ro : ro + gn * 128].reshape(gn, 128, m_out)
        flat[:, off : off + gn * m_out] = (
            blk.transpose(1, 0, 2).reshape(128, gn * m_out).astype(BF)
        )
    tail_arrs = {}
    for si, trows in tails:
        ro = seg_off[si] + (segs[si] // 128) * 128
        if si == last:
            blk = np.concatenate([wt[ro : seg_off[si + 1]], bias], axis=0)
        else:
            blk = wt[ro : ro + trows]
        tail_arrs[si] = np.ascontiguousarray(blk).astype(BF)
    return flat, tail_arrs


def _prep_inputs(inputs):
    """Build the 8 per-core input maps from the full (unsharded) inputs."""
    g = {k: np.asarray(v, F32) for k, v in inputs.items()}

    def ext(v):
        return np.concatenate(
            [np.asarray(v, F32).ravel(), [1.0]]
        ).reshape(-1, 1).astype(BF)

    common = {
        "x5": ext(g["fw_evol_diff"]),
        "x6": ext(g["fw_update_diff"]),
        "obs": ext(np.concatenate([g["obs_diff"], g["obs_innov_diff"]])),
        "h_q": _play_ext(g["h_Q"], 5),
        "h_sig": _play_ext(g["h_Sigma"], 5),
        "h_s": _play_ext(g["h_S"], 5),
        "h_q_f": g["h_Q"].reshape(1, H).copy(),
        "h_sig_f": g["h_Sigma"].reshape(1, H).copy(),
        "h_s_f": g["h_S"].reshape(1, H).copy(),
    }

    wT = {
        "w5": (g["W5"].T, g["b5"]), "w6": (g["W6"].T, g["b6"]),
        "w7": (g["W7"].T, g["b7"]), "w1": (g["W1"].T, g["b1"]),
    }
    for tag, suf in (("q", "Q"), ("sig", "Sig"), ("s", "S")):
        Wih, Whh = g[f"Wih_{suf}"], g[f"Whh_{suf}"]
        bih, bhh = g[f"bih_{suf}"], g[f"bhh_{suf}"]
        wT[f"wrz_{tag}"] = (
            np.concatenate([Wih[0 : 2 * H], Whh[0 : 2 * H]], axis=1).T,
            bih[0 : 2 * H] + bhh[0 : 2 * H],
        )
        wT[f"win_{tag}"] = (Wih[2 * H :].T, bih[2 * H :])
        wT[f"whn_{tag}"] = (Whh[2 * H :].T, bhh[2 * H :])

    for wname, (segs, m_out) in WSPECS.items():
        w, b = wT[wname]
        flat, tails = _pack_w(w, segs, m_out, b)
        if flat is not None:
            common[f"{wname}_f"] = flat
        for si, arr in tails.items():
            common[f"{wname}_t{si}"] = arr

    stripes = _stripes()
    groups = _w2b_groups()
    in_maps = []
    for k in range(NCORES):
        m = dict(common)
        sl = slice(k * MSH, (k + 1) * MSH)
        w2aT = np.ascontiguousarray(g["W2a"][sl, :].T)       # [1152, 5760]
        fulls, merged = [], []
        for s, (m0, nsz) in enumerate(stripes):
            blk = w2aT[np.r_[0:512, 576:1088], m0 : m0 + nsz]
            fulls.append(
                blk.reshape(8, 128, nsz).transpose(1, 0, 2)
                .reshape(128, 8 * nsz).astype(BF)
            )
            merged.append(np.concatenate(
                [w2aT[512:576, m0 : m0 + nsz],
                 w2aT[1088:1152, m0 : m0 + nsz]], axis=0
            ).astype(BF))
        m["w2af"] = np.stack(fulls[:-1])
        m["w2af_t"] = fulls[-1]
        m["w2am"] = np.stack(merged[:-1])
        m["w2am_t"] = merged[-1]
        m["b2a"] = g["b2a"][sl].reshape(1, -1).copy()

        w2bT = np.ascontiguousarray(g["W2b"][:, sl].T)       # [5760, 576]
        w2bG = w2bT.reshape(NM2, 128, D2_OUT)
        w2b_full = np.empty((len(groups) - 1, 128, W2B_GRP * D2_OUT), BF)
        for gi, (kb0, kn) in enumerate(groups[:-1]):
            w2b_full[gi] = (
                w2bG[kb0 : kb0 + kn].transpose(1, 0, 2)
                .reshape(128, kn * D2_OUT).astype(BF)
            )
        kb0, kn = groups[-1]
        m["w2b"] = w2b_full
        m["w2b_t"] = np.ascontiguousarray(
            w2bG[kb0 : kb0 + kn].transpose(1, 0, 2).reshape(128, kn * D2_OUT)
        ).astype(BF)
        in_maps.append(m)
    return in_maps


def run(trace=False, **inputs):
    from concourse.bass_utils import run_bass_kernel_spmd

    nc = _get_program()
    in_maps = _prep_inputs(inputs)
    res = run_bass_kernel_spmd(nc, in_maps, list(range(NCORES)), trace=trace)
    y = np.zeros(D2_OUT, np.float64)
    for r in res.results:
        y += r["y"].reshape(-1).astype(np.float64)
    out = (y.astype(F32) + np.asarray(inputs["b2b"], F32)).reshape(24, 24)
    return out, res


def kernel(**inputs):
    out, _ = run(trace=False, **inputs)
    return out


# revision 26
# speedup vs baseline: 1.2091x; 1.0392x over previous
"""Trainium2 Bass kernel for the KNet-style recurrent chain (batch=1).

Strategy (memory-bound, ~353MB fp32 weights on host):
  - ALL weights are bf16 on device: halves HBM traffic; bf16 moving
    operand runs 1 cycle/row on the PE at any width. Host-sim rel err
    of bf16 weights+activations is 3.9e-3 (gate: 2e-2).
  - Small GRU chain + small FCs REPLICATED on all 8 cores; FC2
    (W2a [46080,1152], W2b [576,46080]) tensor-parallel: each core
    takes 5760 rows of W2a / columns of W2b; host sums the 8 partials.
  - Matvecs run weight-moving on the PE:
        psum[1, N] (+)= x_chunk[K, 1].T @ W.T_chunk[K, N]
  - Chain biases are FOLDED into the weights as one extra K-row; every
    activation vector carries a literal 1.0 at position d. Activations
    (sigmoid/tanh/relu) read PSUM directly - no bias adds.
  - Weights are HOST-PACKED into exact SBUF tile layouts so every DMA
    is contiguous rows of >=2KB (DMA queues are descriptor-rate bound
    below ~2KB/descriptor). Chain groups are 16KB/partition to cut
    per-group semaphore waits.
  - FC2a contracts over [hSig | hS] P-layout chunks (k-split, no in2
    concat). The two 64-row K-tails merge into ONE 128-row chunk whose
    activation column is a free-layout concat + one transpose -> 9
    full chunks per stripe. b2a is added on the vector engine.
  - The FC2 stripe loop is software-pipelined on the PE: stripe s's
    matvec runs while stripe s-1's transposes + FC2b y512 group
    execute, so the add+relu engine hop is off the PE critical path.
    The y[512:576] slice of FC2b accumulates on the vector engine
    (scalar_tensor_tensor) with a final ones-vector matmul reduce.
  - W2a stripe DMAs are issued at chain milestones (6 during the
    chain, 6 during the FC2 phase) so the big weights stream in the
    chain's idle DMA bandwidth without delaying chain weights much.
  - PSUM: every accumulation group gets its own [1,576] slot (tag
    rotation bufs=3, 2 banks each) - no group ever shares a bank, so
    start=True can never clear a co-tenant's has_written state.
    y512/y64 keep dedicated banks (8 banks total).
"""

import sys

sys.path.insert(0, "/opt/trn_rl_repo")

import numpy as np
import ml_dtypes

NCORES = 8
H = 576                       # hidden size of all three GRUs
D2_HID, D2_IN, D2_OUT = 46080, 1152, 576
MSH = D2_HID // NCORES        # 5760 rows of W2a per core
NM2 = MSH // 128              # 45 h_fc columns per core
NSTR = 12                     # FC2a stripes: 11x512 + 1x128
W2B_GRP = 4                   # FC2b k-blocks per group: 45 = 11*4 + 1
CAP = 8192                    # bytes/partition per chain weight group

F32 = np.float32
BF = ml_dtypes.bfloat16

# chain weights: name -> (seg K sizes, m_out). Bias rides on the last
# segment's tail chunk (one extra row).
WSPECS = {
    "w5": ([24], 480), "w6": ([24], 480), "w7": ([48], 960),
    "w1": ([H], H),
    "wrz_q": ([480, H], 1152), "win_q": ([480], H), "whn_q": ([H], H),
    "wrz_sig": ([H, 480, H], 1152), "win_sig": ([H, 480], H),
    "whn_sig": ([H], H),
    "wrz_s": ([H, 960, H], 1152), "win_s": ([H, 960], H),
    "whn_s": ([H], H),
}


def _stripes():
    return [(s * 512, min(512, MSH - s * 512)) for s in range(NSTR)]


def _w2b_groups():
    return [(g * W2B_GRP, min(W2B_GRP, NM2 - g * W2B_GRP))
            for g in range((NM2 + W2B_GRP - 1) // W2B_GRP)]


def _grp(m_out):
    return max(1, CAP // (m_out * 2))


def _wplan(segs, m_out):
    """Deterministic chunk plan shared by builder and host packer."""
    g = _grp(m_out)
    full, tails = [], []
    off = 0
    last = len(segs) - 1
    for si, d in enumerate(segs):
        nb, tail = d // 128, d % 128
        for g0 in range(0, nb, g):
            gn = min(g, nb - g0)
            full.append((si, g0, gn, off))
            off += gn * m_out
        trows = tail + (1 if si == last else 0)
        if trows:
            tails.append((si, trows))
    return full, tails, off


def _ncols(d):
    return (d + 127) // 128


_CACHE = {}


class _Vec:
    """Activation vector in SBUF P-layout [128, ncols], with a literal
    1.0 stored at flat position d (row d%128, col d//128)."""

    def __init__(self, tile, d):
        self.tile = tile
        self.d = d

    def full_chunks(self):
        for c in range(self.d // 128):
            yield self.tile[0:128, c : c + 1]

    def tail_ap(self, trows):
        c = self.d // 128
        return self.tile[0:trows, c : c + 1]


def _build_program():
    import concourse.bass as bass  # noqa: F401
    from concourse import bacc, mybir
    import concourse.tile as tile

    f32 = mybir.dt.float32
    bf16 = mybir.dt.bfloat16
    AF = mybir.ActivationFunctionType
    ALU = mybir.AluOpType

    nc = bacc.Bacc(
        "TRN2", target_bir_lowering=False, debug=False, num_devices=NCORES
    )

    def din(name, shape, dt=bf16):
        return nc.dram_tensor(name, list(shape), dt, kind="ExternalInput")

    # --- activation inputs (extended with the 1.0 bias marker) ---
    d_x5 = din("x5", (25, 1))
    d_x6 = din("x6", (25, 1))
    d_obs = din("obs", (49, 1))
    d_hq = din("h_q", (128, 5))       # P-layout, 1.0 at (64, 4)
    d_hsig = din("h_sig", (128, 5))
    d_hs = din("h_s", (128, 5))
    d_hq_f = din("h_q_f", (1, H), f32)    # free-layout (elementwise)
    d_hsig_f = din("h_sig_f", (1, H), f32)
    d_hs_f = din("h_s_f", (1, H), f32)

    # --- chain weights: host-packed bf16, biases folded ---
    dwf, dwt = {}, {}
    for wname, (segs, m_out) in WSPECS.items():
        full, tails, tot = _wplan(segs, m_out)
        if tot:
            dwf[wname] = din(f"{wname}_f", (128, tot))
        for si, trows in tails:
            dwt[(wname, si)] = din(f"{wname}_t{si}", (trows, m_out))

    # --- FC2 weights: host-packed bf16, stripe/group major ---
    # per stripe: full [128, 8, nsz] (hSig c0-3 + hS c0-3) and merged
    # tail [128, nsz] (hSig rows 512:576 ; hS rows 512:576).
    d_w2af = din("w2af", (NSTR - 1, 128, 8 * 512))
    d_w2af_t = din("w2af_t", (128, 8 * 128))
    d_w2am = din("w2am", (NSTR - 1, 128, 512))
    d_w2am_t = din("w2am_t", (128, 128))
    d_b2a = din("b2a", (1, MSH), f32)
    d_w2b = din("w2b", (len(_w2b_groups()) - 1, 128, W2B_GRP * D2_OUT))
    d_w2b_t = din("w2b_t", (128, 1 * D2_OUT))

    d_y = nc.dram_tensor("y", [1, D2_OUT], f32, kind="ExternalOutput")

    with tile.TileContext(nc) as tc:
        with (
            tc.tile_pool(name="const", bufs=1) as constp,
            tc.tile_pool(name="vecs", bufs=1) as vecp,
            tc.tile_pool(name="smallw", bufs=1) as swp,
            tc.tile_pool(name="w2ap", bufs=1) as w2ap,
            tc.tile_pool(name="w2bp", bufs=1) as w2bp,
            tc.tile_pool(name="ps", bufs=1, space="PSUM") as psp,
        ):
            def load_const(dram, shape, name, dt=bf16):
                t = constp.tile(list(shape), dt, name=name, tag=name)
                nc.sync.dma_start(out=t, in_=dram[:])
                return t

            x5 = _Vec(load_const(d_x5, (25, 1), "t_x5"), 24)
            x6 = _Vec(load_const(d_x6, (25, 1), "t_x6"), 24)
            obs = _Vec(load_const(d_obs, (49, 1), "t_obs"), 48)
            h_q = _Vec(load_const(d_hq, (128, 5), "t_hq"), H)
            h_sig = _Vec(load_const(d_hsig, (128, 5), "t_hsig"), H)
            h_s = _Vec(load_const(d_hs, (128, 5), "t_hs"), H)
            hf = {
                "q": load_const(d_hq_f, (1, H), "t_hq_f", f32),
                "sig": load_const(d_hsig_f, (1, H), "t_hsig_f", f32),
                "s": load_const(d_hs_f, (1, H), "t_hs_f", f32),
            }
            ident = constp.tile([1, 1], f32, name="ident", tag="ident")
            nc.vector.memset(ident, 1.0)
            ones = constp.tile([128, 1], f32, name="ones", tag="ones")
            nc.vector.memset(ones, 1.0)
            h_fc = constp.tile([128, NM2], bf16, name="h_fc", tag="h_fc")
            acc64 = constp.tile([128, 64], f32, name="acc64", tag="acc64")
            nc.vector.memset(acc64, 0.0)

            def ps_alloc(name, tag="mvb", bufs=2):
                return psp.tile([1, 576], f32, name=name, tag=tag,
                                bufs=bufs)

            def ps_t_alloc(name):
                return psp.tile([128, 9], f32, name=name, tag="mvb",
                                bufs=2)

            def matvec(wname, segs, m_out, ps_tag="mvb", ps_bufs=2):
                """PE matvec over P-layout segs (bias folded into the
                weights). Returns [(psum_ap, m0, width)] slot list: each
                576-wide output slot is its own PSUM allocation, so no
                accumulation group ever shares a bank."""
                full, tails, _ = _wplan([v.d for v in segs], m_out)
                chunks = []          # (wt_ap, rhs_ap, ksz)
                for si, g0, gn, foff in full:
                    wt = swp.tile([128, gn, m_out], bf16, tag="sw",
                                  name=f"w_{wname}_{si}_{g0}", bufs=6)
                    nc.sync.dma_start(
                        out=wt, in_=dwf[wname][:, foff : foff + gn * m_out]
                    )
                    fc = list(segs[si].full_chunks())
                    for b in range(gn):
                        chunks.append((wt[:, b, :], fc[g0 + b], 128))
                for si, trows in tails:
                    wtt = swp.tile([trows, m_out], bf16, tag="sw",
                                   name=f"w_{wname}_t{si}", bufs=6)
                    nc.sync.dma_start(out=wtt, in_=dwt[(wname, si)][:])
                    chunks.append((wtt, segs[si].tail_ap(trows), trows))

                nch = len(chunks)
                slots = []
                for m0 in range(0, m_out, 576):
                    w = min(576, m_out - m0)
                    slots.append((ps_alloc(f"ps_{wname}_{m0}", ps_tag,
                                           ps_bufs), m0, w))
                for ci, (wt_ap, rhs, ksz) in enumerate(chunks):
                    for ps, m0, w in slots:
                        for n0 in range(0, w, 512):
                            nsz = min(512, w - n0)
                            nc.tensor.matmul(
                                ps[0:1, n0 : n0 + nsz],
                                rhs,
                                wt_ap[0:ksz, m0 + n0 : m0 + n0 + nsz],
                                start=(ci == 0),
                                stop=(ci == nch - 1),
                                skip_group_check=True,
                            )
                return slots

            def to_play(free_ap, d, name):
                """transpose free-layout [1, d] -> P-layout [128, ncols]
                bf16 tile with a 1.0 planted at position d."""
                n_m = _ncols(d)
                cols = d // 128
                tl = d % 128
                ps_t = ps_t_alloc(f"pst_{name}")
                for c in range(n_m):
                    csz = min(128, d - c * 128)
                    nc.tensor.matmul(
                        ps_t[0:csz, c : c + 1],
                        free_ap[0:1, c * 128 : c * 128 + csz],
                        ident,
                        is_transpose=True,
                        start=(c == 0),
                        stop=(c == n_m - 1),
                        skip_group_check=True,
                    )
                pl = vecp.tile([128, n_m], bf16, name=name, tag=name)
                nc.vector.tensor_copy(pl, ps_t[:, 0:n_m])
                nc.vector.memset(pl[tl : tl + 1, cols : cols + 1], 1.0)
                return _Vec(pl, d)

            def gru(g, x_segs, h, ps_ghn, out_name):
                (ps_r, _, _), (ps_z, _, _) = matvec(
                    f"wrz_{g}", x_segs + [h], 2 * H)
                (ps_gin, _, _), = matvec(f"win_{g}", x_segs, H)
                rz = vecp.tile([1, 2 * H], f32, name=f"rz_{g}", tag="rz")
                nc.scalar.activation(rz[0:1, 0:H], ps_r[0:1, 0:H],
                                     AF.Sigmoid)
                nc.scalar.activation(rz[0:1, H : 2 * H], ps_z[0:1, 0:H],
                                     AF.Sigmoid)
                # n = tanh(gin + r*ghn); h' = (1-z)*n + z*h, with z*h and
                # (1-z) computed on the vector engine while tanh runs.
                t3 = vecp.tile([1, H], f32, name=f"t3_{g}", tag="t3")
                nc.vector.tensor_mul(t3, rz[0:1, 0:H], ps_ghn[0:1, 0:H])
                nc.vector.tensor_add(t3, t3, ps_gin[0:1, 0:H])
                n_t = vecp.tile([1, H], f32, name=f"n_{g}", tag="n_t")
                nc.scalar.activation(n_t, t3, AF.Tanh)
                zh = vecp.tile([1, H], f32, name=f"zh_{g}", tag="zh")
                nc.vector.tensor_mul(zh, rz[0:1, H : 2 * H], hf[g])
                omz = vecp.tile([1, H], f32, name=f"omz_{g}", tag="omz")
                nc.vector.tensor_scalar(omz, rz[0:1, H : 2 * H], -1.0, 1.0,
                                        op0=ALU.mult, op1=ALU.add)
                hn = vecp.tile([1, H], f32, name=out_name, tag=out_name)
                nc.vector.tensor_mul(hn, n_t, omz)
                nc.vector.tensor_add(hn, hn, zh)
                return hn

            def relu_mv(wname, segs, m_out, name):
                slots = matvec(wname, segs, m_out)
                out = vecp.tile([1, m_out], f32, name=name, tag="vf",
                                bufs=2)
                for ps, m0, w in slots:
                    nc.scalar.activation(out[0:1, m0 : m0 + w],
                                         ps[0:1, 0:w], AF.Relu)
                return out

            b2a_sb = constp.tile([1, MSH], f32, name="b2a_sb", tag="b2a")
            nc.sync.dma_start(out=b2a_sb, in_=d_b2a[:])
            groups = _w2b_groups()
            w2a_tiles = []
            stripes_l = _stripes()

            def issue_w2a(n):
                for _ in range(n):
                    s = len(w2a_tiles)
                    if s >= NSTR:
                        return
                    m0, nsz = stripes_l[s]
                    last = nsz != 512
                    wtf = w2ap.tile([128, 8, nsz], bf16, tag="w2a",
                                    name=f"w2af_{s}", bufs=7)
                    nc.sync.dma_start(
                        out=wtf, in_=(d_w2af_t if last else d_w2af[s])[:])
                    wtm = w2ap.tile([128, nsz], bf16, tag="w2am",
                                    name=f"w2am_{s}", bufs=7)
                    nc.sync.dma_start(
                        out=wtm, in_=(d_w2am_t if last else d_w2am[s])[:])
                    w2a_tiles.append((wtf, wtm))

            # ---- the serial chain ----
            # out6/out7 depend only on kernel inputs: compute them first
            # to fill the startup window and empty the inter-GRU gaps.
            out5_f = relu_mv("w5", [x5], 480, "out5_f")
            out5 = to_play(out5_f, 480, "out5")
            out6_f = relu_mv("w6", [x6], 480, "out6_f")
            out6 = to_play(out6_f, 480, "out6")
            out7_f = relu_mv("w7", [obs], 960, "out7_f")
            out7 = to_play(out7_f, 960, "out7")
            # whn matvecs depend only on the constant h states: run each
            # during the PREVIOUS GRU's elementwise window (PE is idle
            # there). Their psums live in a dedicated PSUM tag so the
            # mvb rotation never pins on them.
            (ghn_q, _, _), = matvec("whn_q", [h_q], H, "ghn", 1)
            hQ_f = gru("q", [out5], h_q, ghn_q, "hQ_f")
            (ghn_sig, _, _), = matvec("whn_sig", [h_sig], H, "ghn", 1)
            hQ = to_play(hQ_f, H, "hQ")
            issue_w2a(2)
            hSig_f = gru("sig", [hQ, out6], h_sig, ghn_sig, "hSig_f")
            (ghn_s, _, _), = matvec("whn_s", [h_s], H, "ghn", 1)
            hSig = to_play(hSig_f, H, "hSig")
            issue_w2a(1)
            out1_f = relu_mv("w1", [hSig], H, "out1_f")
            out1 = to_play(out1_f, H, "out1")
            issue_w2a(2)
            hS_f = gru("s", [out1, out7], h_s, ghn_s, "hS_f")
            hS = to_play(hS_f, H, "hS")
            issue_w2a(12)

            # merged FC2a tail column: [hSig 512:576 ; hS 512:576]
            tmf = vecp.tile([1, 128], f32, name="tails_f", tag="tails_f")
            nc.vector.tensor_copy(tmf[0:1, 0:64], hSig_f[0:1, 512:576])
            nc.vector.tensor_copy(tmf[0:1, 64:128], hS_f[0:1, 512:576])
            ps_m = ps_t_alloc("pst_merge")
            nc.tensor.matmul(ps_m[0:128, 0:1], tmf, ident,
                             is_transpose=True, start=True, stop=True,
                             skip_group_check=True)
            tails_pl = vecp.tile([128, 1], bf16, name="tails_pl",
                                 tag="tails_pl")
            nc.vector.tensor_copy(tails_pl, ps_m[:, 0:1])

            # ---- FC2a stripes + FC2b, software-pipelined on the PE:
            # stripe s's matvec runs while stripe s-1's transposes and
            # FC2b group execute (hstr hop latency is off the PE path).
            hsig_cols = list(hSig.full_chunks())
            hs_cols = list(hS.full_chunks())
            ps_y512 = psp.tile([1, 512], f32, name="ps_y512", tag="y512",
                               bufs=1)
            ps_y64 = psp.tile([1, 64], f32, name="ps_y64", tag="y64",
                              bufs=1)
            w2b_tiles = {}
            hstrs = {}

            def fc2b_group(g):
                kb0, kn = groups[g]
                wtg = w2b_tiles[g]
                for j in range(kn):
                    kb = kb0 + j
                    nc.tensor.matmul(
                        ps_y512[0:1, :], h_fc[:, kb : kb + 1],
                        wtg[:, j, 0:512],
                        start=(kb == 0), stop=(kb == NM2 - 1),
                        skip_group_check=True,
                    )
                    nc.vector.scalar_tensor_tensor(
                        acc64, wtg[:, j, 512:576],
                        h_fc[:, kb : kb + 1], acc64,
                        op0=ALU.mult, op1=ALU.add,
                    )

            def finish_stripe(s):
                m0, nsz = stripes_l[s]
                ncol = nsz // 128
                hstr = hstrs.pop(s)
                ps_t = ps_t_alloc(f"pst_s{s}")
                for c in range(ncol):
                    nc.tensor.matmul(
                        ps_t[:, c : c + 1],
                        hstr[0:1, c * 128 : (c + 1) * 128],
                        ident,
                        is_transpose=True,
                        start=(c == 0),
                        stop=(c == ncol - 1),
                        skip_group_check=True,
                    )
                col0 = m0 // 128
                nc.vector.tensor_copy(
                    h_fc[:, col0 : col0 + ncol], ps_t[:, 0:ncol]
                )
                fc2b_group(s)

            for s, (m0, nsz) in enumerate(_stripes()):
                wtf, wtm = w2a_tiles[s]
                # prefetch this stripe's w2b group
                kb0, kn = groups[s]
                wtg = w2bp.tile([128, kn, D2_OUT], bf16, tag="w2b",
                                name=f"w2b_{s}", bufs=4)
                nc.sync.dma_start(
                    out=wtg, in_=(d_w2b[s] if kn == W2B_GRP
                                  else d_w2b_t)[:])
                w2b_tiles[s] = wtg

                psf = ps_alloc(f"ps_f{s}")
                rhs_list = (
                    [(wtf[:, c, :], hsig_cols[c], 128) for c in range(4)]
                    + [(wtf[:, 4 + c, :], hs_cols[c], 128) for c in range(4)]
                    + [(wtm, tails_pl[0:128, 0:1], 128)]
                )
                for ci, (wt_ap, rhs, ksz) in enumerate(rhs_list):
                    nc.tensor.matmul(
                        psf[0:1, 0:nsz],
                        rhs,
                        wt_ap[0:ksz, 0:nsz],
                        start=(ci == 0),
                        stop=(ci == 8),
                        skip_group_check=True,
                    )
                hstr = vecp.tile([1, 512], f32, name=f"hstr_{s}",
                                 tag="hstr", bufs=2)
                nc.vector.tensor_add(hstr[0:1, 0:nsz], psf[0:1, 0:nsz],
                                     b2a_sb[0:1, m0 : m0 + nsz])
                nc.scalar.activation(hstr[0:1, 0:nsz], hstr[0:1, 0:nsz],
                                     AF.Relu)
                hstrs[s] = hstr
                if s > 0:
                    finish_stripe(s - 1)
            finish_stripe(NSTR - 1)

            nc.tensor.matmul(ps_y64[0:1, :], ones, acc64,
                             start=True, stop=True, skip_group_check=True)
            y_sb = constp.tile([1, D2_OUT], f32, name="y_sb", tag="y_sb")
            nc.vector.tensor_copy(y_sb[:, 0:512], ps_y512)
            nc.vector.tensor_copy(y_sb[:, 512:576], ps_y64)
            nc.sync.dma_start(out=d_y[:], in_=y_sb)

    nc.compile()
    return nc


def _get_program():
    if "nc" not in _CACHE:
        _CACHE["nc"] = _build_program()
    return _CACHE["nc"]


# ----------------------------------------------------------------------------
# host-side data prep
# ----------------------------------------------------------------------------


def _play_ext(v, ncols):
    """vector + trailing 1.0 -> P-layout [128, ncols] bf16."""
    v = np.concatenate([np.asarray(v, F32).ravel(), [1.0]])
    buf = np.zeros((ncols, 128), F32)
    buf.reshape(-1)[: v.size] = v
    return np.ascontiguousarray(buf.T).astype(BF)


def _pack_w(wt, segs, m_out, bias):
    """Pack W.T [K, m_out] fp32 + bias into (flat [128, tot] bf16,
    {seg_idx: tail bf16}); bias row on the last segment's tail."""
    full, tails, tot = _wplan(segs, m_out)
    wt = np.asarray(wt, F32)
    bias = np.asarray(bias, F32).reshape(1, m_out)
    flat = np.empty((128, tot), BF) if tot else None
    seg_off = np.concatenate([[0], np.cumsum(segs)]).astype(int)
    last = len(segs) - 1
    for si, g0, gn, off in full:
        ro = seg_off[si] + g0 * 128
        blk = wt[ro : ro + gn * 128].reshape(gn, 128, m_out)
        flat[:, off : off + gn * m_out] = (
            blk.transpose(1, 0, 2).reshape(128, gn * m_out).astype(BF)
        )
    tail_arrs = {}
    for si, trows in tails:
        ro = seg_off[si] + (segs[si] // 128) * 128
        if si == last:
            blk = np.concatenate([wt[ro : seg_off[si + 1]], bias], axis=0)
        else:
            blk = wt[ro : ro + trows]
        tail_arrs[si] = np.ascontiguousarray(blk).astype(BF)
    return flat, tail_arrs


def _prep_inputs(inputs):
    """Build the 8 per-core input maps from the full (unsharded) inputs."""
    g = {k: np.asarray(v, F32) for k, v in inputs.items()}

    def ext(v):
        return np.concatenate(
            [np.asarray(v, F32).ravel(), [1.0]]
        ).reshape(-1, 1).astype(BF)

    common = {
        "x5": ext(g["fw_evol_diff"]),
        "x6": ext(g["fw_update_diff"]),
        "obs": ext(np.concatenate([g["obs_diff"], g["obs_innov_diff"]])),
        "h_q": _play_ext(g["h_Q"], 5),
        "h_sig": _play_ext(g["h_Sigma"], 5),
        "h_s": _play_ext(g["h_S"], 5),
        "h_q_f": g["h_Q"].reshape(1, H).copy(),
        "h_sig_f": g["h_Sigma"].reshape(1, H).copy(),
        "h_s_f": g["h_S"].reshape(1, H).copy(),
    }

    wT = {
        "w5": (g["W5"].T, g["b5"]), "w6": (g["W6"].T, g["b6"]),
        "w7": (g["W7"].T, g["b7"]), "w1": (g["W1"].T, g["b1"]),
    }
    for tag, suf in (("q", "Q"), ("sig", "Sig"), ("s", "S")):
        Wih, Whh = g[f"Wih_{suf}"], g[f"Whh_{suf}"]
        bih, bhh = g[f"bih_{suf}"], g[f"bhh_{suf}"]
        wT[f"wrz_{tag}"] = (
            np.concatenate([Wih[0 : 2 * H], Whh[0 : 2 * H]], axis=1).T,
            bih[0 : 2 * H] + bhh[0 : 2 * H],
        )
        wT[f"win_{tag}"] = (Wih[2 * H :].T, bih[2 * H :])
        wT[f"whn_{tag}"] = (Whh[2 * H :].T, bhh[2 * H :])

    for wname, (segs, m_out) in WSPECS.items():
        w, b = wT[wname]
        flat, tails = _pack_w(w, segs, m_out, b)
        if flat is not None:
            common[f"{wname}_f"] = flat
        for si, arr in tails.items():
            common[f"{wname}_t{si}"] = arr

    stripes = _stripes()
    groups = _w2b_groups()
    in_maps = []
    for k in range(NCORES):
        m = dict(common)
        sl = slice(k * MSH, (k + 1) * MSH)
        w2aT = np.ascontiguousarray(g["W2a"][sl, :].T)       # [1152, 5760]
        fulls, merged = [], []
        for s, (m0, nsz) in enumerate(stripes):
            blk = w2aT[np.r_[0:512, 576:1088], m0 : m0 + nsz]
            fulls.append(
                blk.reshape(8, 128, nsz).transpose(1, 0, 2)
                .reshape(128, 8 * nsz).astype(BF)
            )
            merged.append(np.concatenate(
                [w2aT[512:576, m0 : m0 + nsz],
                 w2aT[1088:1152, m0 : m0 + nsz]], axis=0
            ).astype(BF))
        m["w2af"] = np.stack(fulls[:-1])
        m["w2af_t"] = fulls[-1]
        m["w2am"] = np.stack(merged[:-1])
        m["w2am_t"] = merged[-1]
        m["b2a"] = g["b2a"][sl].reshape(1, -1).copy()

        w2bT = np.ascontiguousarray(g["W2b"][:, sl].T)       # [5760, 576]
        w2bG = w2bT.reshape(NM2, 128, D2_OUT)
        w2b_full = np.empty((len(groups) - 1, 128, W2B_GRP * D2_OUT), BF)
        for gi, (kb0, kn) in enumerate(groups[:-1]):
            w2b_full[gi] = (
                w2bG[kb0 : kb0 + kn].transpose(1, 0, 2)
                .reshape(128, kn * D2_OUT).astype(BF)
            )
        kb0, kn = groups[-1]
        m["w2b"] = w2b_full
        m["w2b_t"] = np.ascontiguousarray(
            w2bG[kb0 : kb0 + kn].transpose(1, 0, 2).reshape(128, kn * D2_OUT)
        ).astype(BF)
        in_maps.append(m)
    return in_maps


def run(trace=False, **inputs):
    from concourse.bass_utils import run_bass_kernel_spmd

    nc = _get_program()
    in_maps = _prep_inputs(inputs)
    res = run_bass_kernel_spmd(nc, in_maps, list(range(NCORES)), trace=trace)
    y = np.zeros(D2_OUT, np.float64)
    for r in res.results:
        y += r["y"].reshape(-1).astype(np.float64)
    out = (y.astype(F32) + np.asarray(inputs["b2b"], F32)).reshape(24, 24)
    return out, res


def kernel(**inputs):
    out, _ = run(trace=False, **inputs)
    return out


# revision 28
# speedup vs baseline: 1.2480x; 1.0322x over previous
"""Trainium2 Bass kernel for the KNet-style recurrent chain (batch=1).

Strategy (memory-bound, ~353MB fp32 weights on host):
  - ALL weights are bf16 on device: halves HBM traffic; bf16 moving
    operand runs 1 cycle/row on the PE at any width. Host-sim rel err
    of bf16 weights+activations is 3.9e-3 (gate: 2e-2).
  - Small GRU chain + small FCs REPLICATED on all 8 cores; FC2
    (W2a [46080,1152], W2b [576,46080]) tensor-parallel: each core
    takes 5760 rows of W2a / columns of W2b; host sums the 8 partials.
  - Matvecs run weight-moving on the PE:
        psum[1, N] (+)= x_chunk[K, 1].T @ W.T_chunk[K, N]
  - Chain biases are FOLDED into the weights as one extra K-row; every
    activation vector carries a literal 1.0 at position d. Activations
    (sigmoid/tanh/relu) read PSUM directly - no bias adds.
  - Weights are HOST-PACKED into exact SBUF tile layouts so every DMA
    is contiguous rows of >=2KB (DMA queues are descriptor-rate bound
    below ~2KB/descriptor). Chain groups are 16KB/partition to cut
    per-group semaphore waits.
  - FC2a contracts over [hSig | hS] P-layout chunks (k-split, no in2
    concat). The two 64-row K-tails merge into ONE 128-row chunk whose
    activation column is a free-layout concat + one transpose -> 9
    full chunks per stripe. b2a is added on the vector engine.
  - The FC2 stripe loop is software-pipelined on the PE: stripe s's
    matvec runs while stripe s-1's transposes + FC2b y512 group
    execute, so the add+relu engine hop is off the PE critical path.
    The y[512:576] slice of FC2b accumulates on the vector engine
    (scalar_tensor_tensor) with a final ones-vector matmul reduce.
  - W2a stripe DMAs are issued at chain milestones (6 during the
    chain, 6 during the FC2 phase) so the big weights stream in the
    chain's idle DMA bandwidth without delaying chain weights much.
  - PSUM: every accumulation group gets its own [1,576] slot (tag
    rotation bufs=3, 2 banks each) - no group ever shares a bank, so
    start=True can never clear a co-tenant's has_written state.
    y512/y64 keep dedicated banks (8 banks total).
"""

import sys

sys.path.insert(0, "/opt/trn_rl_repo")

import numpy as np
import ml_dtypes

NCORES = 8
H = 576                       # hidden size of all three GRUs
D2_HID, D2_IN, D2_OUT = 46080, 1152, 576
MSH = D2_HID // NCORES        # 5760 rows of W2a per core
NM2 = MSH // 128              # 45 h_fc columns per core
NSTR = 12                     # FC2a stripes: 11x512 + 1x128
W2B_GRP = 4                   # FC2b k-blocks per group: 45 = 11*4 + 1
CAP = 8192                    # bytes/partition per chain weight group

F32 = np.float32
BF = ml_dtypes.bfloat16

# chain weights: name -> (seg K sizes, m_out). Bias rides on the last
# segment's tail chunk (one extra row).
WSPECS = {
    "w5": ([24], 480), "w6": ([24], 480), "w7": ([48], 960),
    "w1": ([H], H),
    "wrz_q": ([480, H], 1152), "win_q": ([480], H), "whn_q": ([H], H),
    "wrz_sig": ([480, H, H], 1152), "win_sig": ([480, H], H),
    "whn_sig": ([H], H),
    "wrz_s": ([960, H, H], 1152), "win_s": ([960, H], H),
    "whn_s": ([H], H),
}


def _stripes():
    return [(s * 512, min(512, MSH - s * 512)) for s in range(NSTR)]


def _w2b_groups():
    return [(g * W2B_GRP, min(W2B_GRP, NM2 - g * W2B_GRP))
            for g in range((NM2 + W2B_GRP - 1) // W2B_GRP)]


def _grp(m_out):
    return max(1, CAP // (m_out * 2))


def _wplan(segs, m_out):
    """Deterministic chunk plan shared by builder and host packer."""
    g = _grp(m_out)
    full, tails = [], []
    off = 0
    last = len(segs) - 1
    for si, d in enumerate(segs):
        nb, tail = d // 128, d % 128
        for g0 in range(0, nb, g):
            gn = min(g, nb - g0)
            full.append((si, g0, gn, off))
            off += gn * m_out
        trows = tail + (1 if si == last else 0)
        if trows:
            tails.append((si, trows))
    return full, tails, off


def _ncols(d):
    return (d + 127) // 128


_CACHE = {}


class _Vec:
    """Activation vector in SBUF P-layout [128, ncols], with a literal
    1.0 stored at flat position d (row d%128, col d//128)."""

    def __init__(self, tile, d):
        self.tile = tile
        self.d = d

    def full_chunks(self):
        for c in range(self.d // 128):
            yield self.tile[0:128, c : c + 1]

    def tail_ap(self, trows):
        c = self.d // 128
        return self.tile[0:trows, c : c + 1]


def _build_program():
    import concourse.bass as bass  # noqa: F401
    from concourse import bacc, mybir
    import concourse.tile as tile

    f32 = mybir.dt.float32
    bf16 = mybir.dt.bfloat16
    AF = mybir.ActivationFunctionType
    ALU = mybir.AluOpType

    nc = bacc.Bacc(
        "TRN2", target_bir_lowering=False, debug=False, num_devices=NCORES
    )

    def din(name, shape, dt=bf16):
        return nc.dram_tensor(name, list(shape), dt, kind="ExternalInput")

    # --- activation inputs (extended with the 1.0 bias marker) ---
    d_x5 = din("x5", (25, 1))
    d_x6 = din("x6", (25, 1))
    d_obs = din("obs", (49, 1))
    d_hq = din("h_q", (128, 5))       # P-layout, 1.0 at (64, 4)
    d_hsig = din("h_sig", (128, 5))
    d_hs = din("h_s", (128, 5))
    d_hq_f = din("h_q_f", (1, H), f32)    # free-layout (elementwise)
    d_hsig_f = din("h_sig_f", (1, H), f32)
    d_hs_f = din("h_s_f", (1, H), f32)

    # --- chain weights: host-packed bf16, biases folded ---
    dwf, dwt = {}, {}
    for wname, (segs, m_out) in WSPECS.items():
        full, tails, tot = _wplan(segs, m_out)
        if tot:
            dwf[wname] = din(f"{wname}_f", (128, tot))
        for si, trows in tails:
            dwt[(wname, si)] = din(f"{wname}_t{si}", (trows, m_out))

    # --- FC2 weights: host-packed bf16, stripe/group major ---
    # per stripe: full [128, 8, nsz] (hSig c0-3 + hS c0-3) and merged
    # tail [128, nsz] (hSig rows 512:576 ; hS rows 512:576).
    d_w2af = din("w2af", (NSTR - 1, 128, 8 * 512))
    d_w2af_t = din("w2af_t", (128, 8 * 128))
    d_w2am = din("w2am", (NSTR - 1, 128, 512))
    d_w2am_t = din("w2am_t", (128, 128))
    d_b2a = din("b2a", (1, MSH), f32)
    d_w2b = din("w2b", (len(_w2b_groups()) - 1, 128, W2B_GRP * D2_OUT))
    d_w2b_t = din("w2b_t", (128, 1 * D2_OUT))

    d_y = nc.dram_tensor("y", [1, D2_OUT], f32, kind="ExternalOutput")

    with tile.TileContext(nc) as tc:
        with (
            tc.tile_pool(name="const", bufs=1) as constp,
            tc.tile_pool(name="vecs", bufs=1) as vecp,
            tc.tile_pool(name="smallw", bufs=1) as swp,
            tc.tile_pool(name="w2ap", bufs=1) as w2ap,
            tc.tile_pool(name="w2bp", bufs=1) as w2bp,
            tc.tile_pool(name="ps", bufs=1, space="PSUM") as psp,
        ):
            def load_const(dram, shape, name, dt=bf16):
                t = constp.tile(list(shape), dt, name=name, tag=name)
                nc.sync.dma_start(out=t, in_=dram[:])
                return t

            x5 = _Vec(load_const(d_x5, (25, 1), "t_x5"), 24)
            x6 = _Vec(load_const(d_x6, (25, 1), "t_x6"), 24)
            obs = _Vec(load_const(d_obs, (49, 1), "t_obs"), 48)
            h_q = _Vec(load_const(d_hq, (128, 5), "t_hq"), H)
            h_sig = _Vec(load_const(d_hsig, (128, 5), "t_hsig"), H)
            h_s = _Vec(load_const(d_hs, (128, 5), "t_hs"), H)
            hf = {
                "q": load_const(d_hq_f, (1, H), "t_hq_f", f32),
                "sig": load_const(d_hsig_f, (1, H), "t_hsig_f", f32),
                "s": load_const(d_hs_f, (1, H), "t_hs_f", f32),
            }
            ident = constp.tile([1, 1], f32, name="ident", tag="ident")
            nc.vector.memset(ident, 1.0)
            ones = constp.tile([128, 1], f32, name="ones", tag="ones")
            nc.vector.memset(ones, 1.0)
            h_fc = constp.tile([128, NM2], bf16, name="h_fc", tag="h_fc")
            acc64 = constp.tile([128, 64], f32, name="acc64", tag="acc64")
            nc.vector.memset(acc64, 0.0)

            def ps_alloc(name, tag="mvb", bufs=2):
                return psp.tile([1, 576], f32, name=name, tag=tag,
                                bufs=bufs)

            def ps_t_alloc(name):
                return psp.tile([128, 9], f32, name=name, tag="mvb",
                                bufs=2)

            def matvec(wname, segs, m_out, ps_tag="mvb", ps_bufs=2):
                """PE matvec over P-layout segs (bias folded into the
                weights). Returns [(psum_ap, m0, width)] slot list: each
                576-wide output slot is its own PSUM allocation, so no
                accumulation group ever shares a bank."""
                full, tails, _ = _wplan([v.d for v in segs], m_out)
                chunks = []          # (wt_ap, rhs_ap, ksz)
                for si, g0, gn, foff in full:
                    wt = swp.tile([128, gn, m_out], bf16, tag="sw",
                                  name=f"w_{wname}_{si}_{g0}", bufs=6)
                    nc.sync.dma_start(
                        out=wt, in_=dwf[wname][:, foff : foff + gn * m_out]
                    )
                    fc = list(segs[si].full_chunks())
                    for b in range(gn):
                        chunks.append((wt[:, b, :], fc[g0 + b], 128))
                for si, trows in tails:
                    wtt = swp.tile([trows, m_out], bf16, tag="sw",
                                   name=f"w_{wname}_t{si}", bufs=6)
                    nc.sync.dma_start(out=wtt, in_=dwt[(wname, si)][:])
                    chunks.append((wtt, segs[si].tail_ap(trows), trows))

                nch = len(chunks)
                slots = []
                for m0 in range(0, m_out, 576):
                    w = min(576, m_out - m0)
                    slots.append((ps_alloc(f"ps_{wname}_{m0}", ps_tag,
                                           ps_bufs), m0, w))
                for ci, (wt_ap, rhs, ksz) in enumerate(chunks):
                    for ps, m0, w in slots:
                        for n0 in range(0, w, 512):
                            nsz = min(512, w - n0)
                            nc.tensor.matmul(
                                ps[0:1, n0 : n0 + nsz],
                                rhs,
                                wt_ap[0:ksz, m0 + n0 : m0 + n0 + nsz],
                                start=(ci == 0),
                                stop=(ci == nch - 1),
                                skip_group_check=True,
                            )
                return slots

            def to_play(free_ap, d, name):
                """transpose free-layout [1, d] -> P-layout [128, ncols]
                bf16 tile with a 1.0 planted at position d."""
                n_m = _ncols(d)
                cols = d // 128
                tl = d % 128
                ps_t = ps_t_alloc(f"pst_{name}")
                for c in range(n_m):
                    csz = min(128, d - c * 128)
                    nc.tensor.matmul(
                        ps_t[0:csz, c : c + 1],
                        free_ap[0:1, c * 128 : c * 128 + csz],
                        ident,
                        is_transpose=True,
                        start=(c == 0),
                        stop=(c == n_m - 1),
                        skip_group_check=True,
                    )
                pl = vecp.tile([128, n_m], bf16, name=name, tag=name)
                nc.vector.tensor_copy(pl, ps_t[:, 0:n_m])
                nc.vector.memset(pl[tl : tl + 1, cols : cols + 1], 1.0)
                return _Vec(pl, d)

            def gru(g, rz_segs, gin_segs, ps_ghn, out_name):
                (ps_r, _, _), (ps_z, _, _) = matvec(
                    f"wrz_{g}", rz_segs, 2 * H)
                (ps_gin, _, _), = matvec(f"win_{g}", gin_segs, H)
                rz = vecp.tile([1, 2 * H], f32, name=f"rz_{g}", tag="rz")
                nc.scalar.activation(rz[0:1, 0:H], ps_r[0:1, 0:H],
                                     AF.Sigmoid)
                nc.scalar.activation(rz[0:1, H : 2 * H], ps_z[0:1, 0:H],
                                     AF.Sigmoid)
                # n = tanh(gin + r*ghn); h' = (1-z)*n + z*h, with z*h and
                # (1-z) computed on the vector engine while tanh runs.
                t3 = vecp.tile([1, H], f32, name=f"t3_{g}", tag="t3")
                nc.vector.tensor_mul(t3, rz[0:1, 0:H], ps_ghn[0:1, 0:H])
                nc.vector.tensor_add(t3, t3, ps_gin[0:1, 0:H])
                n_t = vecp.tile([1, H], f32, name=f"n_{g}", tag="n_t")
                nc.scalar.activation(n_t, t3, AF.Tanh)
                zh = vecp.tile([1, H], f32, name=f"zh_{g}", tag="zh")
                nc.vector.tensor_mul(zh, rz[0:1, H : 2 * H], hf[g])
                omz = vecp.tile([1, H], f32, name=f"omz_{g}", tag="omz")
                nc.vector.tensor_scalar(omz, rz[0:1, H : 2 * H], -1.0, 1.0,
                                        op0=ALU.mult, op1=ALU.add)
                hn = vecp.tile([1, H], f32, name=out_name, tag=out_name)
                nc.vector.tensor_mul(hn, n_t, omz)
                nc.vector.tensor_add(hn, hn, zh)
                return hn

            def relu_mv(wname, segs, m_out, name):
                slots = matvec(wname, segs, m_out)
                out = vecp.tile([1, m_out], f32, name=name, tag="vf",
                                bufs=2)
                for ps, m0, w in slots:
                    nc.scalar.activation(out[0:1, m0 : m0 + w],
                                         ps[0:1, 0:w], AF.Relu)
                return out

            b2a_sb = constp.tile([1, MSH], f32, name="b2a_sb", tag="b2a")
            nc.sync.dma_start(out=b2a_sb, in_=d_b2a[:])
            groups = _w2b_groups()
            w2a_tiles = []
            stripes_l = _stripes()

            def issue_w2a(n):
                for _ in range(n):
                    s = len(w2a_tiles)
                    if s >= NSTR:
                        return
                    m0, nsz = stripes_l[s]
                    last = nsz != 512
                    wtf = w2ap.tile([128, 8, nsz], bf16, tag="w2a",
                                    name=f"w2af_{s}", bufs=7)
                    nc.sync.dma_start(
                        out=wtf, in_=(d_w2af_t if last else d_w2af[s])[:])
                    wtm = w2ap.tile([128, nsz], bf16, tag="w2am",
                                    name=f"w2am_{s}", bufs=7)
                    nc.sync.dma_start(
                        out=wtm, in_=(d_w2am_t if last else d_w2am[s])[:])
                    w2a_tiles.append((wtf, wtm))

            # ---- the serial chain ----
            # out6/out7 depend only on kernel inputs: compute them first
            # to fill the startup window and empty the inter-GRU gaps.
            out5_f = relu_mv("w5", [x5], 480, "out5_f")
            out5 = to_play(out5_f, 480, "out5")
            out6_f = relu_mv("w6", [x6], 480, "out6_f")
            out6 = to_play(out6_f, 480, "out6")
            out7_f = relu_mv("w7", [obs], 960, "out7_f")
            out7 = to_play(out7_f, 960, "out7")
            # whn matvecs depend only on the constant h states: run each
            # during the PREVIOUS GRU's elementwise window (PE is idle
            # there). Their psums live in a dedicated PSUM tag so the
            # mvb rotation never pins on them.
            (ghn_q, _, _), = matvec("whn_q", [h_q], H, "ghn", 1)
            hQ_f = gru("q", [out5, h_q], [out5], ghn_q, "hQ_f")
            (ghn_sig, _, _), = matvec("whn_sig", [h_sig], H, "ghn", 1)
            hQ = to_play(hQ_f, H, "hQ")
            issue_w2a(2)
            hSig_f = gru("sig", [out6, h_sig, hQ], [out6, hQ],
                         ghn_sig, "hSig_f")
            (ghn_s, _, _), = matvec("whn_s", [h_s], H, "ghn", 1)
            hSig = to_play(hSig_f, H, "hSig")
            issue_w2a(1)
            out1_f = relu_mv("w1", [hSig], H, "out1_f")
            out1 = to_play(out1_f, H, "out1")
            issue_w2a(2)
            hS_f = gru("s", [out7, h_s, out1], [out7, out1],
                       ghn_s, "hS_f")
            hS = to_play(hS_f, H, "hS")
            issue_w2a(12)

            # merged FC2a tail column: [hSig 512:576 ; hS 512:576]
            tmf = vecp.tile([1, 128], f32, name="tails_f", tag="tails_f")
            nc.vector.tensor_copy(tmf[0:1, 0:64], hSig_f[0:1, 512:576])
            nc.vector.tensor_copy(tmf[0:1, 64:128], hS_f[0:1, 512:576])
            ps_m = ps_t_alloc("pst_merge")
            nc.tensor.matmul(ps_m[0:128, 0:1], tmf, ident,
                             is_transpose=True, start=True, stop=True,
                             skip_group_check=True)
            tails_pl = vecp.tile([128, 1], bf16, name="tails_pl",
                                 tag="tails_pl")
            nc.vector.tensor_copy(tails_pl, ps_m[:, 0:1])

            # ---- FC2a stripes + FC2b, software-pipelined on the PE:
            # stripe s's matvec runs while stripe s-1's transposes and
            # FC2b group execute (hstr hop latency is off the PE path).
            hsig_cols = list(hSig.full_chunks())
            hs_cols = list(hS.full_chunks())
            ps_y512 = psp.tile([1, 512], f32, name="ps_y512", tag="y512",
                               bufs=1)
            ps_y64 = psp.tile([1, 64], f32, name="ps_y64", tag="y64",
                              bufs=1)
            w2b_tiles = {}
            hstrs = {}

            def fc2b_group(g):
                kb0, kn = groups[g]
                wtg = w2b_tiles[g]
                for j in range(kn):
                    kb = kb0 + j
                    nc.tensor.matmul(
                        ps_y512[0:1, :], h_fc[:, kb : kb + 1],
                        wtg[:, j, 0:512],
                        start=(kb == 0), stop=(kb == NM2 - 1),
                        skip_group_check=True,
                    )
                    nc.vector.scalar_tensor_tensor(
                        acc64, wtg[:, j, 512:576],
                        h_fc[:, kb : kb + 1], acc64,
                        op0=ALU.mult, op1=ALU.add,
                    )

            def finish_stripe(s):
                m0, nsz = stripes_l[s]
                ncol = nsz // 128
                hstr = hstrs.pop(s)
                ps_t = ps_t_alloc(f"pst_s{s}")
                for c in range(ncol):
                    nc.tensor.matmul(
                        ps_t[:, c : c + 1],
                        hstr[0:1, c * 128 : (c + 1) * 128],
                        ident,
                        is_transpose=True,
                        start=(c == 0),
                        stop=(c == ncol - 1),
                        skip_group_check=True,
                    )
                col0 = m0 // 128
                nc.vector.tensor_copy(
                    h_fc[:, col0 : col0 + ncol], ps_t[:, 0:ncol]
                )
                fc2b_group(s)

            for s, (m0, nsz) in enumerate(_stripes()):
                wtf, wtm = w2a_tiles[s]
                # prefetch this stripe's w2b group
                kb0, kn = groups[s]
                wtg = w2bp.tile([128, kn, D2_OUT], bf16, tag="w2b",
                                name=f"w2b_{s}", bufs=4)
                nc.sync.dma_start(
                    out=wtg, in_=(d_w2b[s] if kn == W2B_GRP
                                  else d_w2b_t)[:])
                w2b_tiles[s] = wtg

                psf = ps_alloc(f"ps_f{s}")
                rhs_list = (
                    [(wtf[:, c, :], hsig_cols[c], 128) for c in range(4)]
                    + [(wtf[:, 4 + c, :], hs_cols[c], 128) for c in range(4)]
                    + [(wtm, tails_pl[0:128, 0:1], 128)]
                )
                for ci, (wt_ap, rhs, ksz) in enumerate(rhs_list):
                    nc.tensor.matmul(
                        psf[0:1, 0:nsz],
                        rhs,
                        wt_ap[0:ksz, 0:nsz],
                        start=(ci == 0),
                        stop=(ci == 8),
                        skip_group_check=True,
                    )
                hstr = vecp.tile([1, 512], f32, name=f"hstr_{s}",
                                 tag="hstr", bufs=2)
                nc.vector.tensor_add(hstr[0:1, 0:nsz], psf[0:1, 0:nsz],
                                     b2a_sb[0:1, m0 : m0 + nsz])
                nc.scalar.activation(hstr[0:1, 0:nsz], hstr[0:1, 0:nsz],
                                     AF.Relu)
                hstrs[s] = hstr
                if s > 0:
                    finish_stripe(s - 1)
            finish_stripe(NSTR - 1)

            nc.tensor.matmul(ps_y64[0:1, :], ones, acc64,
                             start=True, stop=True, skip_group_check=True)
            y_sb = constp.tile([1, D2_OUT], f32, name="y_sb", tag="y_sb")
            nc.vector.tensor_copy(y_sb[:, 0:512], ps_y512)
            nc.vector.tensor_copy(y_sb[:, 512:576], ps_y64)
            nc.sync.dma_start(out=d_y[:], in_=y_sb)

    nc.compile()
    return nc


def _get_program():
    if "nc" not in _CACHE:
        _CACHE["nc"] = _build_program()
    return _CACHE["nc"]


# ----------------------------------------------------------------------------
# host-side data prep
# ----------------------------------------------------------------------------


def _play_ext(v, ncols):
    """vector + trailing 1.0 -> P-layout [128, ncols] bf16."""
    v = np.concatenate([np.asarray(v, F32).ravel(), [1.0]])
    buf = np.zeros((ncols, 128), F32)
    buf.reshape(-1)[: v.size] = v
    return np.ascontiguousarray(buf.T).astype(BF)


def _pack_w(wt, segs, m_out, bias):
    """Pack W.T [K, m_out] fp32 + bias into (flat [128, tot] bf16,
    {seg_idx: tail bf16}); bias row on the last segment's tail."""
    full, tails, tot = _wplan(segs, m_out)
    wt = np.asarray(wt, F32)
    bias = np.asarray(bias, F32).reshape(1, m_out)
    flat = np.empty((128, tot), BF) if tot else None
    seg_off = np.concatenate([[0], np.cumsum(segs)]).astype(int)
    last = len(segs) - 1
    for si, g0, gn, off in full:
        ro = seg_off[si] + g0 * 128
        blk = wt[ro : ro + gn * 128].reshape(gn, 128, m_out)
        flat[:, off : off + gn * m_out] = (
            blk.transpose(1, 0, 2).reshape(128, gn * m_out).astype(BF)
        )
    tail_arrs = {}
    for si, trows in tails:
        ro = seg_off[si] + (segs[si] // 128) * 128
        if si == last:
            blk = np.concatenate([wt[ro : seg_off[si + 1]], bias], axis=0)
        else:
            blk = wt[ro : ro + trows]
        tail_arrs[si] = np.ascontiguousarray(blk).astype(BF)
    return flat, tail_arrs


def _prep_inputs(inputs):
    """Build the 8 per-core input maps from the full (unsharded) inputs."""
    g = {k: np.asarray(v, F32) for k, v in inputs.items()}

    def ext(v):
        return np.concatenate(
            [np.asarray(v, F32).ravel(), [1.0]]
        ).reshape(-1, 1).astype(BF)

    common = {
        "x5": ext(g["fw_evol_diff"]),
        "x6": ext(g["fw_update_diff"]),
        "obs": ext(np.concatenate([g["obs_diff"], g["obs_innov_diff"]])),
        "h_q": _play_ext(g["h_Q"], 5),
        "h_sig": _play_ext(g["h_Sigma"], 5),
        "h_s": _play_ext(g["h_S"], 5),
        "h_q_f": g["h_Q"].reshape(1, H).copy(),
        "h_sig_f": g["h_Sigma"].reshape(1, H).copy(),
        "h_s_f": g["h_S"].reshape(1, H).copy(),
    }

    wT = {
        "w5": (g["W5"].T, g["b5"]), "w6": (g["W6"].T, g["b6"]),
        "w7": (g["W7"].T, g["b7"]), "w1": (g["W1"].T, g["b1"]),
    }
    for tag, suf in (("q", "Q"), ("sig", "Sig"), ("s", "S")):
        Wih, Whh = g[f"Wih_{suf}"], g[f"Whh_{suf}"]
        bih, bhh = g[f"bih_{suf}"], g[f"bhh_{suf}"]
        wT[f"wrz_{tag}"] = (
            np.concatenate([Wih[0 : 2 * H], Whh[0 : 2 * H]], axis=1).T,
            bih[0 : 2 * H] + bhh[0 : 2 * H],
        )
        wT[f"win_{tag}"] = (Wih[2 * H :].T, bih[2 * H :])
        wT[f"whn_{tag}"] = (Whh[2 * H :].T, bhh[2 * H :])

    # permute W.T rows to the kernel's seg order (ready segs first)
    PERms = {
        "wrz_sig": [(576, 1056), (1056, 1632), (0, 576)],
        "win_sig": [(576, 1056), (0, 576)],
        "wrz_s": [(576, 1536), (1536, 2112), (0, 576)],
        "win_s": [(576, 1536), (0, 576)],
    }
    for nm, ranges in PERms.items():
        w, b = wT[nm]
        wT[nm] = (np.concatenate([w[a:z] for a, z in ranges], axis=0), b)

    for wname, (segs, m_out) in WSPECS.items():
        w, b = wT[wname]
        flat, tails = _pack_w(w, segs, m_out, b)
        if flat is not None:
            common[f"{wname}_f"] = flat
        for si, arr in tails.items():
            common[f"{wname}_t{si}"] = arr

    stripes = _stripes()
    groups = _w2b_groups()
    in_maps = []
    for k in range(NCORES):
        m = dict(common)
        sl = slice(k * MSH, (k + 1) * MSH)
        w2aT = np.ascontiguousarray(g["W2a"][sl, :].T)       # [1152, 5760]
        fulls, merged = [], []
        for s, (m0, nsz) in enumerate(stripes):
            blk = w2aT[np.r_[0:512, 576:1088], m0 : m0 + nsz]
            fulls.append(
                blk.reshape(8, 128, nsz).transpose(1, 0, 2)
                .reshape(128, 8 * nsz).astype(BF)
            )
            merged.append(np.concatenate(
                [w2aT[512:576, m0 : m0 + nsz],
                 w2aT[1088:1152, m0 : m0 + nsz]], axis=0
            ).astype(BF))
        m["w2af"] = np.stack(fulls[:-1])
        m["w2af_t"] = fulls[-1]
        m["w2am"] = np.stack(merged[:-1])
        m["w2am_t"] = merged[-1]
        m["b2a"] = g["b2a"][sl].reshape(1, -1).copy()

        w2bT = np.ascontiguousarray(g["W2b"][:, sl].T)       # [5760, 576]
        w2bG = w2bT.reshape(NM2, 128, D2_OUT)
        w2b_full = np.empty((len(groups) - 1, 128, W2B_GRP * D2_OUT), BF)
        for gi, (kb0, kn) in enumerate(groups[:-1]):
            w2b_full[gi] = (
                w2bG[kb0 : kb0 + kn].transpose(1, 0, 2)
                .reshape(128, kn * D2_OUT).astype(BF)
            )
        kb0, kn = groups[-1]
        m["w2b"] = w2b_full
        m["w2b_t"] = np.ascontiguousarray(
            w2bG[kb0 : kb0 + kn].transpose(1, 0, 2).reshape(128, kn * D2_OUT)
        ).astype(BF)
        in_maps.append(m)
    return in_maps


def run(trace=False, **inputs):
    from concourse.bass_utils import run_bass_kernel_spmd

    nc = _get_program()
    in_maps = _prep_inputs(inputs)
    res = run_bass_kernel_spmd(nc, in_maps, list(range(NCORES)), trace=trace)
    y = np.zeros(D2_OUT, np.float64)
    for r in res.results:
        y += r["y"].reshape(-1).astype(np.float64)
    out = (y.astype(F32) + np.asarray(inputs["b2b"], F32)).reshape(24, 24)
    return out, res


def kernel(**inputs):
    out, _ = run(trace=False, **inputs)
    return out
